# revision 36
# baseline (speedup 1.0000x reference)
"""PreT_Attention (prefix-KV multi-head attention) on 8 Trainium2 NeuronCores.

Strategy: pure data parallelism — batch B=8 is split 1 element per core; the
qkv/proj weights are replicated. No collectives. Host-side numpy does layout
marshalling only (transposes / reshapes), all FLOPs run on device.

Device kernel (per core), bf16 operands with f32 PSUM accumulation. The
schedule is a single software-pipelined stream built around the PE engine
(the roofline engine at ~150us of matmul column-cycles):

  - DMA: few large partition-major transfers (issue cost on a sequencer is
    ~1us each, so tensors are packed host-side for one DMA apiece). The
    bootstrap weights (k0/q0 column blocks of W_qkv) are a separate small
    tensor so the first matmuls start ~3us in. Nothing is issued on the
    scalar queue — DMA issues there would delay the exp stream.
  - qkv projection tiles (q^T / k^T pair tiles, natural-v tiles) from
    pre-transposed x^T and W^T with the contraction dim on partitions. Only
    pair 0's q/k tiles are produced up front; v tiles interleave with head
    0's S/PV loop; later q/k pair tiles are emitted as chunk-granular FILLER
    inside the attention pipeline so the PE stays busy during act-bound
    stretches.
  - attention: flat cross-head software pipeline — the S/exp stream runs 2
    steps ahead of the PV stream (across head boundaries), so the act engine
    never faces a cold e0 at head entry. The prefix-KV part of S is computed
    once per head pair from a block-diagonal packed k-prefix (one exp for
    both heads); softmax denominators ride as a 65th ones-column per head
    block in v (zeroed on the other head's rows in the packed v-prefix).
  - normalization: DVE copies PSUM out fast (frees the single 'o' bank pair
    in ~1.2us), then reciprocal + gpsimd partition_broadcast + DVE mul into
    A^T. The last head uses a shorter chain (reciprocal straight from PSUM,
    broadcast via a PE matmul with a ones stationary) since the PE idles
    there and the proj tail gates on it.
  - output projection: kt=5 (the last-normalized pair) is deferred behind
    other tiles' kt0-4 accumulation; proj tile 0's mains pre-run as pair-5
    filler. With the common all-zero bias the PSUM->SBUF y copy runs on
    the act engine (idle during proj; keeps the DVE off the psum-reuse
    WAR chain); a nonzero bias falls back to a DVE add against a
    pre-broadcast [128,C] bias tile.

The m (key/value position) axis is ordered [tokens(1024) | prefix(64)] —
softmax is permutation invariant, and this keeps every tile 128-aligned.
"""

import os
import sys

if os.environ.get("PRET_NOCACHE"):
    try:
        import jax
        jax.config.update("jax_enable_compilation_cache", False)
    except Exception:
        pass

for _p in ("/opt/trn_rl_repo", "/root/.axon_site/_ro/trn_rl_repo"):
    if os.path.isdir(_p) and _p not in sys.path:
        sys.path.insert(0, _p)

import numpy as np

import contextlib
import itertools

import concourse.bass as bass
import concourse.mybir as mybir
import concourse.tile as tile
from concourse import bacc
from concourse import bass_utils
from concourse import library_config
from concourse.bass_utils import run_bass_kernel_spmd

F32 = mybir.dt.float32
R32 = mybir.dt.float32r
BF16 = mybir.dt.bfloat16
EXP = mybir.ActivationFunctionType.Exp

B, N, C, H, D, P = 8, 1024, 768, 12, 64, 64
M = N + P            # 1088 key/value positions, tokens first then prefix
KT = C // 128        # 6 contraction k-tiles
NT = N // 128        # 8 token tiles
MT = N // 128        # 8 full (token) m-tiles; prefix handled separately
NP = H // 2          # 6 head pairs
WR = 16 * 128        # wq_rest columns per kt: q1-5 | k1-5 | v(6)
SCALE = D ** -0.5

REPEAT = int(os.environ.get("PRET_REPEAT", "1"))
DT = {"fp32r": R32, "bf16": BF16}[os.environ.get("PRET_DT", "bf16")]
POLICY = os.environ.get("PRET_POLICY", "3")
GDO = os.environ.get("PRET_GDO", "0")
# filler chunks pulled per pipeline slot in the attention steady loop
PULL = int(os.environ.get("PRET_PULL", "1"))
LEAD_N = int(os.environ.get("PRET_LEAD", "2"))
EB = int(os.environ.get("PRET_EB", "3"))


@contextlib.contextmanager
def _ldw_opt():
    # Patches walrus flags: --policy (post-scheduler) and --enable-ldw-opt
    # (fp32r only; bf16 matmuls emit explicit InstLdweights that walrus
    # rejects under ldw-opt).
    if DT != R32 and POLICY == "0" and GDO == "0":
        yield
        return
    orig = bass_utils.run_command

    def patched(argv, **kw):
        out = []
        for a in argv:
            if a == "--enable-ldw-opt=false" and DT == R32:
                a = "--enable-ldw-opt=true"
            elif a == "--policy=0":
                a = f"--policy={POLICY}"
            out.append(a)
        if GDO != "0" and out and out[0].endswith("walrus_driver"):
            out.append(f"--global-dma-ordering-optimization={GDO}")
        if os.environ.get("PRET_SDMA_SP") and out and out[0].endswith("walrus_driver"):
            out = [a.replace("--assign-static-dmas-to-sp=false",
                             "--assign-static-dmas-to-sp=true") for a in out]
        return orig(out, **kw)

    bass_utils.run_command = patched
    try:
        yield
    finally:
        bass_utils.run_command = orig


def build_nc(repeat=REPEAT, zero_bias=bool(int(os.environ.get("PRET_ZB", "1")))):
    nc = bacc.Bacc("TRN2", target_bir_lowering=False, debug=False,
                   dynamic_dma_scratch_size=int(os.environ.get("PRET_DDS", "16384")),
                   use_seq_codegen=bool(int(os.environ.get("PRET_SEQCG", "0"))),
                   num_swdge_queues=int(os.environ.get("PRET_SWQ", "1")))

    # all inputs partition-major so each is a single DMA
    xT0 = nc.dram_tensor("xT0", (128, N), DT, kind="ExternalInput")
    xTr = nc.dram_tensor("xTr", (128, (KT - 1) * N), DT, kind="ExternalInput")
    # bootstrap W columns: per kt [q-pair0 (128) | k-pair0 (128)]
    wqb = nc.dram_tensor("wqb", (128, KT * 256), DT, kind="ExternalInput")
    # the rest: per kt [q pairs 1-5 (640) | k pairs 1-5 (640) | v (768)]
    wqr = nc.dram_tensor("wqr", (128, KT * WR), DT, kind="ExternalInput")
    wp = nc.dram_tensor("wp", (128, KT * C), DT, kind="ExternalInput")
    # block-diag packed prefix k per head pair: [[k_even^T, 0], [0, k_odd^T]]
    kpk = nc.dram_tensor("kpk", (128, NP * 128), DT, kind="ExternalInput")
    # packed prefix v (+ ones col) per pair/head, other head's rows zeroed
    vpk = nc.dram_tensor("vpk", (128, NP * 2 * (D + 1)), DT, kind="ExternalInput")
    bb = nc.dram_tensor("bb", (128, C), DT, kind="ExternalInput")  # bias bcast
    y = nc.dram_tensor("y", (NT, 128, C), DT, kind="ExternalOutput")
    DEBUG = bool(os.environ.get("PRET_DEBUG"))
    if DEBUG:
        qdbg = nc.dram_tensor("qdbg", (128, KT * N), DT, kind="ExternalOutput")
        kdbg = nc.dram_tensor("kdbg", (128, KT * N), DT, kind="ExternalOutput")
        adbg = nc.dram_tensor("adbg", (128, KT * N), DT, kind="ExternalOutput")
        vdbg = nc.dram_tensor("vdbg", (128, MT * H * 65), DT, kind="ExternalOutput")

    with tile.TileContext(nc) as tc:
        with (
            nc.allow_low_precision(reason="bf16/fp32r matmul operands, f32 accum"),
            tc.tile_pool(name="const", bufs=1) as const_pool,
            tc.tile_pool(name="data", bufs=1) as data_pool,
            tc.tile_pool(name="work", bufs=2) as work_pool,
            tc.tile_pool(name="psum", bufs=2, space="PSUM") as pp,
        ):
            # ---- persistent SBUF tensors ----
            q_sb = data_pool.tile([128, KT, N], DT)          # q^T, pair rows
            ktok_sb = data_pool.tile([128, KT, N], DT)       # k^T tokens, pair rows
            v_sb = data_pool.tile([128, MT, H * 65], DT)     # v + ones cols
            a_sb = data_pool.tile([128, KT, N], DT)          # A^T attn out
            # input tiles double-buffered across repeat bodies: body i+1's
            # DMAs land in the other parity set, so they never WAR-wait on
            # body i's late filler reads (which run ~95% into the body)
            kpk_sbs = [data_pool.tile([128, NP, 128], DT, name=f"kpk{p}") for p in range(2)]
            vpk_sbs = [data_pool.tile([128, NP, 2, D + 1], DT, name=f"vpk{p}") for p in range(2)]
            wp_sbs = [data_pool.tile([128, KT, C], DT, name=f"wp{p}") for p in range(2)]
            bb_sbs = [data_pool.tile([128, C], DT, name=f"bb{p}") for p in range(2)]
            xT0_sbs = [data_pool.tile([128, N], DT, name=f"xT0{p}") for p in range(2)]
            xTr_sbs = [data_pool.tile([128, KT - 1, N], DT, name=f"xTr{p}") for p in range(2)]
            wqb_sbs = [data_pool.tile([128, KT, 2, 128], DT, name=f"wqb{p}") for p in range(2)]
            wqr_sbs = [data_pool.tile([128, KT, WR], DT, name=f"wqr{p}") for p in range(2)]

            nc.gpsimd.load_library(library_config.attn)
            # memset can't write fp32r/bf16; stage ones in f32 and copy.
            # col 64 of each head block of v must be 1.0 (softmax denoms)
            ones_f32 = const_pool.tile([128, 128], F32)
            nc.vector.memset(ones_f32[:], 1.0)
            ones_dt = const_pool.tile([1, 128], DT)
            nc.vector.tensor_copy(ones_dt[:], ones_f32[0:1, :])
            v_ones = v_sb.rearrange("p m (h e) -> p m h e", e=65)[:, :, :, 64]
            nc.vector.tensor_copy(
                v_ones, ones_f32[:, 0 : MT * H].rearrange("p (m h) -> p m h", m=MT)
            )

            def emit_body(par=0):
              kpk_sb, vpk_sb, wp_sb, bb_sb = (
                  kpk_sbs[par], vpk_sbs[par], wp_sbs[par], bb_sbs[par])
              xT0_sb, xTr_sb, wqb_sb, wqr_sb = (
                  xT0_sbs[par], xTr_sbs[par], wqb_sbs[par], wqr_sbs[par])

              def xt(kt):
                  return xT0_sb[:] if kt == 0 else xTr_sb[:, kt - 1, :]

              def wq_col(kt, mt):
                  # stationary W^T column block for qk out-tile mt (0..11)
                  if mt == 0:
                      return wqb_sb[:, kt, 0, :]
                  if mt == KT:
                      return wqb_sb[:, kt, 1, :]
                  if mt < KT:   # q pairs 1..5
                      c0 = (mt - 1) * 128
                  else:         # k pairs 1..5
                      c0 = 640 + (mt - KT - 1) * 128
                  return wqr_sb[:, kt, c0 : c0 + 128]

              def wv_col(kt, j0, j1):
                  return wqr_sb[:, kt, 1280 + j0 : 1280 + j1]

              # ---- DMA: one transfer per tensor; nothing on the act queue ----
              nc.sync.dma_start(xT0_sb[:], xT0[:])
              nc.sync.dma_start(wqb_sb.rearrange("p t h c -> p (t h c)"), wqb[:])
              nc.sync.dma_start(
                  xTr_sb[:, 0:2, :].rearrange("p t n -> p (t n)"),
                  xTr[:, 0 : 2 * N],
              )
              nc.sync.dma_start(
                  xTr_sb[:, 2:, :].rearrange("p t n -> p (t n)"),
                  xTr[:, 2 * N :],
              )
              nc.gpsimd.dma_start(kpk_sb.rearrange("p t m -> p (t m)"), kpk[:])
              nc.gpsimd.dma_start(vpk_sb.rearrange("p t h e -> p (t h e)"), vpk[:])
              nc.gpsimd.dma_start(wqr_sb.rearrange("p t c -> p (t c)"), wqr[:])
              nc.gpsimd.dma_start(wp_sb.rearrange("p t c -> p (t c)"), wp[:])
              if not zero_bias:
                  nc.gpsimd.dma_start(bb_sb[:], bb[:])

              tile_seq = itertools.count()

              def qk_emitter(mt, tag="f", bufs=1):
                  # generator: one 512-col matmul chunk per next(); the
                  # PSUM->SBUF copy runs when the generator is exhausted
                  ps = pp.tile([128, N], F32, tag=tag, bufs=bufs, name=f"ps_qk{mt}")
                  k0 = next(tile_seq)
                  for i in range(KT):
                      kt = (k0 + i) % KT
                      for nb in range(2):
                          nc.tensor.matmul(
                              ps[:, nb * 512 : (nb + 1) * 512],
                              wq_col(kt, mt),
                              xt(kt)[:, nb * 512 : (nb + 1) * 512],
                              start=(i == 0),
                              stop=(i == KT - 1),
                          )
                          yield
                  if mt < KT:
                      nc.vector.tensor_copy(q_sb[:, mt, :], ps[:])
                  else:
                      nc.vector.tensor_copy(ktok_sb[:, mt - KT, :], ps[:])

              def drain(gen):
                  for _ in gen:
                      pass

              def emit_v_tile(nt, tag="f", bufs=1):
                  # n-tile nt of natural v: stationary x^T, moving W_v^T
                  ps = pp.tile([128, 1024], F32, tag=tag, bufs=bufs, name=f"ps_v{nt}")
                  k0 = next(tile_seq)
                  for i in range(KT):
                      kt = (k0 + i) % KT
                      for j0, j1 in ((0, 512), (512, C)):
                          nc.tensor.matmul(
                              ps[:, j0:j1],
                              xt(kt)[:, nt * 128 : (nt + 1) * 128],
                              wv_col(kt, j0, j1),
                              start=(i == 0),
                              stop=(i == KT - 1),
                          )
                  dst = v_sb.rearrange("p m (h e) -> p m h e", e=65)[:, nt, :, 0:D]
                  nc.vector.tensor_copy(dst, ps[:, 0:C].rearrange("p (h d) -> p h d", h=H))

              # ---- attention helpers ----
              def s_pref(t):
                  # packed prefix S for both heads of pair t: one [128,1024]
                  # tile (rows 0:64 = even head's 64 prefix keys, 64:128 odd)
                  ps = pp.tile([128, N], F32, tag="s", name=f"ps_sp{t}")
                  for nb in range(2):
                      nc.tensor.matmul(
                          ps[:, nb * 512 : (nb + 1) * 512],
                          kpk_sb[:, t, :],
                          q_sb[:, t, nb * 512 : (nb + 1) * 512],
                          start=True,
                          stop=True,
                      )
                  ep = work_pool.tile([128, N], DT, tag="ep", bufs=2, name=f"ep{t}")
                  nc.scalar.activation(ep[:], ps[:], EXP, scale=SCALE)
                  return ep

              def emit_s(h, mt):
                  t, r = h // 2, (h % 2) * 64
                  ps = pp.tile([128, N], F32, tag="s", name=f"ps_s{h}_{mt}")
                  for nb in range(2):
                      nc.tensor.matmul(
                          ps[:, nb * 512 : (nb + 1) * 512],
                          ktok_sb[r : r + D, t, mt * 128 : (mt + 1) * 128],
                          q_sb[r : r + D, t, nb * 512 : (nb + 1) * 512],
                          start=True,
                          stop=True,
                      )
                  e_sb = work_pool.tile([128, N], DT, tag="e", bufs=EB, name=f"e{h}_{mt}")
                  nc.scalar.activation(e_sb[:], ps[:], EXP, scale=SCALE)
                  return e_sb

              def emit_pv_pref(h, po, ep):
                  # prefix PV: stationary has the other head's rows zeroed
                  t = h // 2
                  for nb in range(2):
                      nc.tensor.matmul(
                          po[:, nb * 512 : (nb + 1) * 512],
                          vpk_sb[:, t, h % 2, :],
                          ep[:, nb * 512 : (nb + 1) * 512],
                          start=True,
                          stop=False,
                      )

              def emit_pv(h, mt, po, e_sb):
                  for nb in range(2):
                      nc.tensor.matmul(
                          po[:, nb * 512 : (nb + 1) * 512],
                          v_sb[:, mt, h * 65 : (h + 1) * 65],
                          e_sb[:, nb * 512 : (nb + 1) * 512],
                          start=False,
                          stop=(mt == MT - 1),
                      )

              def normalize(h, po):
                  # Copy po out first ([65,1024] DVE copy) so the single 'o'
                  # PSUM buffer frees in ~1.2us instead of holding through the
                  # full recip->broadcast->mul chain (~3.7us).
                  t, r = h // 2, (h % 2) * 64
                  o_sb = work_pool.tile([65, N], F32, tag="oc", bufs=2, name=f"oc{h}")
                  nc.vector.tensor_copy(o_sb[:], po[:])
                  r_sb = work_pool.tile([1, N], F32, tag="r", name=f"r{h}")
                  nc.vector.reciprocal(r_sb[:], o_sb[64:65, :])
                  rb_sb = work_pool.tile([64, N], F32, tag="rb", name=f"rb{h}")
                  nc.gpsimd.partition_broadcast(rb_sb[:], r_sb[:])
                  nc.vector.tensor_mul(a_sb[r : r + 64, t, :], o_sb[0:64, :], rb_sb[:])

              def normalize_last(h, po):
                  # short-latency tail for the final head: reciprocal straight
                  # from PSUM (bf16 out), broadcast via a PE ones-matmul (the
                  # PE idles here; saves the ~1.5us gpsimd hop), multiply
                  # straight from PSUM.
                  t, r = h // 2, (h % 2) * 64
                  r_bf = work_pool.tile([1, N], DT, tag="r2", bufs=1, name=f"r2{h}")
                  nc.vector.reciprocal(r_bf[:], po[64:65, :])
                  o_sb = work_pool.tile([65, N], F32, tag="oc", bufs=2, name=f"oc{h}")
                  nc.vector.tensor_copy(o_sb[0:64, :], po[0:64, :])
                  rb_ps = pp.tile([128, N], F32, tag="s", name=f"rb_ps{h}")
                  for nb in range(2):
                      nc.tensor.matmul(
                          rb_ps[0:64, nb * 512 : (nb + 1) * 512],
                          ones_dt[0:1, 0:64],
                          r_bf[0:1, nb * 512 : (nb + 1) * 512],
                          start=True,
                          stop=True,
                      )
                  nc.vector.tensor_mul(
                      a_sb[r : r + 64, t, :], o_sb[0:64, :], rb_ps[0:64, :]
                  )

              def pull(stream, n=PULL):
                  for _ in range(n):
                      if next(stream, StopIteration) is StopIteration:
                          return

              def vtag(nt):
                  # alternate v tiles between the 'f' and 's' psum tags so the
                  # copy-drain of tile nt-1 never gates tile nt's first matmul
                  return ("f", 1) if nt % 2 == 0 else ("s", 2)

              # ---- pair 0 bootstrap: k0/q0 up front, v tiles inside head 0 ----
              drain(qk_emitter(KT + 0, tag="s", bufs=2))   # k pair 0
              drain(qk_emitter(0, tag="s", bufs=2))        # q pair 0
              eps = {0: s_pref(0)}
              h = 0
              po = pp.tile([65, N], F32, tag="o", bufs=1, name="ps_o0")
              emit_v_tile(0, *vtag(0))
              es0 = [emit_s(h, 0), emit_s(h, 1)]
              emit_pv_pref(h, po, eps[0])
              emit_pv(h, 0, po, es0[0])
              for mt in range(2, MT):
                  emit_v_tile(mt - 1, *vtag(mt - 1))
                  es0.append(emit_s(h, mt))
                  emit_pv(h, mt - 1, po, es0[mt - 1])
              emit_v_tile(MT - 1, *vtag(MT - 1))
              emit_pv(h, MT - 1, po, es0[MT - 1])
              normalize(h, po)

              # proj psum tiles: allocated by chunk generators, finished later
              pys = {}

              def proj_main_chunks(nt, tag, bufs):
                  # kt = 0..4 accumulation of proj tile nt (kt=5 needs the
                  # last pair's A and is deferred to proj_finish)
                  py = pp.tile([128, 1024], F32, tag=tag, bufs=bufs,
                               name=f"ps_y{nt}")
                  pys[nt] = py
                  for kt in range(KT - 1):
                      for j0, j1 in ((0, 512), (512, C)):
                          nc.tensor.matmul(
                              py[:, j0:j1],
                              a_sb[:, kt, nt * 128 : (nt + 1) * 128],
                              wp_sb[:, kt, j0:j1],
                              start=(kt == 0),
                              stop=False,
                          )
                          yield

              # ---- heads 1..11: flat cross-head software pipeline ----
              LEAD = LEAD_N
              flat = [(hh, mm) for hh in range(1, H) for mm in range(MT)]
              es = {}
              pos = {}
              streams = {0: itertools.chain(qk_emitter(1), qk_emitter(KT + 1))}
              for t in range(1, NP - 1):
                  streams[t] = itertools.chain(
                      qk_emitter(t + 1), qk_emitter(KT + t + 1)
                  )
              # pair 5 has no next pair to produce; pre-run proj tile 0's
              # kt0-4 chunks there instead (they only need pairs 0..4)
              streams[NP - 1] = proj_main_chunks(0, "f", 1)

              def s_step(i):
                  if i >= len(flat):
                      return
                  hs, ms = flat[i]
                  if ms == 0 and hs % 2 == 0:
                      # the S stream is about to read pair hs//2's q/k tiles:
                      # force any unfinished producer chunks (and their
                      # PSUM->SBUF copies) to emit BEFORE the first read
                      if hs // 2 - 1 in streams:
                          drain(streams[hs // 2 - 1])
                      eps[hs // 2] = s_pref(hs // 2)
                  es[(hs, ms)] = emit_s(hs, ms)

              # prime the S stream
              for i in range(LEAD):
                  s_step(i)
              for i, (hh, mm) in enumerate(flat):
                  stream = streams[hh // 2]
                  s_step(i + LEAD)
                  pull(stream)
                  if mm == 0:
                      po = pp.tile([65, N], F32, tag="o", bufs=1, name=f"ps_o{hh}")
                      pos[hh] = po
                      emit_pv_pref(hh, po, eps[hh // 2])
                  emit_pv(hh, mm, po, es.pop((hh, mm)))
                  pull(stream)
                  if mm == MT - 1:
                      if hh == H - 1:
                          normalize_last(hh, pos.pop(hh))
                      else:
                          normalize(hh, pos.pop(hh))
                      if hh % 2 == 1:
                          drain(streams[hh // 2])

              # ---- output projection + bias (bias via DVE add) ----
              # kt=5 (head pair 5, the last to normalize) is deferred ~2
              # tiles behind the kt=0..4 accumulation so the last normalize
              # hides behind other proj work. Tile 0's mains already ran as
              # pair-5 filler. Tag sequence keeps every psum-buf reuse behind
              # the finish that drains it.
              def proj_finish(nt):
                  py = pys.pop(nt)
                  kt = KT - 1
                  for j0, j1 in ((0, 512), (512, C)):
                      nc.tensor.matmul(
                          py[:, j0:j1],
                          a_sb[:, kt, nt * 128 : (nt + 1) * 128],
                          wp_sb[:, kt, j0:j1],
                          start=False,
                          stop=True,
                      )
                  y_sb = work_pool.tile([128, C], DT, tag="y", name=f"y{nt}")
                  if zero_bias:
                      nc.scalar.activation(
                          y_sb[:], py[:, 0:C], mybir.ActivationFunctionType.Copy
                      )
                  else:
                      nc.vector.tensor_add(y_sb[:], py[:, 0:C], bb_sb[:])
                  nc.sync.dma_start(y[nt], y_sb[:])

              for step in (
                  ("m", 1, "s", 2), ("m", 2, "s", 2), ("f", 0),
                  ("m", 3, "f", 1), ("f", 1), ("m", 4, "s", 2), ("f", 2),
                  ("m", 5, "s", 2), ("f", 3), ("m", 6, "f", 1), ("f", 4),
                  ("m", 7, "s", 2), ("f", 5), ("f", 6), ("f", 7),
              ):
                  if step[0] == "m":
                      drain(proj_main_chunks(step[1], step[2], step[3]))
                  else:
                      proj_finish(step[1])

              if DEBUG:
                  nc.sync.dma_start(qdbg[:], q_sb.rearrange("p t n -> p (t n)"))
                  nc.sync.dma_start(kdbg[:], ktok_sb.rearrange("p t n -> p (t n)"))
                  nc.sync.dma_start(adbg[:], a_sb.rearrange("p t n -> p (t n)"))
                  nc.sync.dma_start(vdbg[:], v_sb.rearrange("p m c -> p (m c)"))

            for rep in range(repeat):
                emit_body(rep % 2)

    nc.compile()
    return nc


def make_in_maps(x, prompt, W_qkv, W_proj, b_proj):
    if DT == BF16:
        import ml_dtypes
        cast = lambda a: np.asarray(a, dtype=ml_dtypes.bfloat16)
    else:
        cast = lambda a: np.asarray(a, dtype=np.float32)
    x, prompt, W_qkv, W_proj, b_proj = map(cast, (x, prompt, W_qkv, W_proj, b_proj))
    npdt = x.dtype
    # W^T with contraction on partitions: [KT, 128, 3C] -> partition-major
    wq_t = W_qkv.T.reshape(KT, 128, 3 * C).transpose(1, 0, 2)  # [128, KT, 3C]
    # bootstrap: q-pair0 cols (0:128) and k-pair0 cols (768:896)
    wqb_h = np.ascontiguousarray(
        np.stack([wq_t[:, :, 0:128], wq_t[:, :, 768:896]], axis=2).reshape(
            128, KT * 256
        )
    )
    # rest: q pairs 1-5 | k pairs 1-5 | v
    wqr_h = np.ascontiguousarray(
        np.concatenate(
            [wq_t[:, :, 128:768], wq_t[:, :, 896:1536], wq_t[:, :, 1536:2304]],
            axis=2,
        ).reshape(128, KT * WR)
    )
    wp_h = np.ascontiguousarray(
        W_proj.T.reshape(KT, 128, C).transpose(1, 0, 2).reshape(128, KT * C)
    )
    bb_h = np.ascontiguousarray(np.broadcast_to(b_proj.reshape(1, C), (128, C)))
    maps = []
    for b in range(B):
        kp = prompt[b, 0]  # (P, H, D)
        vp = prompt[b, 1]  # (P, H, D)
        kpk_h = np.zeros((128, NP, 128), dtype=npdt)
        vpk_h = np.zeros((128, NP, 2, D + 1), dtype=npdt)
        for t in range(NP):
            kpk_h[0:D, t, 0:P] = kp[:, 2 * t, :].T
            kpk_h[D:128, t, P:128] = kp[:, 2 * t + 1, :].T
            vpk_h[0:P, t, 0, 0:D] = vp[:, 2 * t, :]
            vpk_h[0:P, t, 0, D] = 1.0
            vpk_h[P:128, t, 1, 0:D] = vp[:, 2 * t + 1, :]
            vpk_h[P:128, t, 1, D] = 1.0
        xt_t = x[b].T.reshape(KT, 128, N)  # [KT, 128, N]
        maps.append(
            {
                "xT0": np.ascontiguousarray(xt_t[0]),
                "xTr": np.ascontiguousarray(
                    xt_t[1:].transpose(1, 0, 2).reshape(128, (KT - 1) * N)
                ),
                "wqb": wqb_h,
                "wqr": wqr_h,
                "wp": wp_h,
                "kpk": np.ascontiguousarray(kpk_h.reshape(128, NP * 128)),
                "vpk": np.ascontiguousarray(vpk_h.reshape(128, NP * 2 * (D + 1))),
                "bb": bb_h,
            }
        )
    return maps


def kernel(x, prompt, W_qkv, W_proj, b_proj, **run_kwargs):
    x = np.asarray(x, dtype=np.float32)
    prompt = np.asarray(prompt, dtype=np.float32)
    W_qkv = np.asarray(W_qkv, dtype=np.float32)
    W_proj = np.asarray(W_proj, dtype=np.float32)
    b_proj = np.asarray(b_proj, dtype=np.float32)

    nc = build_nc(repeat=1, zero_bias=bool(np.all(b_proj == 0)))
    in_maps = make_in_maps(x, prompt, W_qkv, W_proj, b_proj)
    with _ldw_opt():
        res = run_bass_kernel_spmd(nc, in_maps, core_ids=list(range(B)), **run_kwargs)
    out = np.stack(
        [np.asarray(res.results[b]["y"], dtype=np.float32).reshape(N, C) for b in range(B)]
    )
    if run_kwargs:
        kernel.last_results = res
    return out


# revision 37
# speedup vs baseline: 1.0163x; 1.0163x over previous
"""PreT_Attention (prefix-KV multi-head attention) on 8 Trainium2 NeuronCores.

Strategy: pure data parallelism — batch B=8 is split 1 element per core; the
qkv/proj weights are replicated. No collectives. Host-side numpy does layout
marshalling only (transposes / reshapes), all FLOPs run on device.

Device kernel (per core), bf16 operands with f32 PSUM accumulation. The
schedule is a single software-pipelined stream built around the PE engine
(the roofline engine at ~150us of matmul column-cycles):

  - DMA: few large partition-major transfers (issue cost on a sequencer is
    ~1us each, so tensors are packed host-side for one DMA apiece). The
    bootstrap weights (k0/q0 column blocks of W_qkv) are a separate small
    tensor so the first matmuls start ~3us in. Nothing is issued on the
    scalar queue — DMA issues there would delay the exp stream.
  - qkv projection tiles (q^T / k^T pair tiles, natural-v tiles) from
    pre-transposed x^T and W^T with the contraction dim on partitions. Only
    pair 0's q/k tiles are produced up front; v tiles interleave with head
    0's S/PV loop; later q/k pair tiles are emitted as chunk-granular FILLER
    inside the attention pipeline so the PE stays busy during act-bound
    stretches.
  - attention: flat cross-head software pipeline — the S/exp stream runs 2
    steps ahead of the PV stream (across head boundaries), so the act engine
    never faces a cold e0 at head entry. The prefix-KV part of S is computed
    once per head pair from a block-diagonal packed k-prefix (one exp for
    both heads); softmax denominators ride as a 65th ones-column per head
    block in v (zeroed on the other head's rows in the packed v-prefix).
  - normalization: DVE copies PSUM out fast (frees the single 'o' bank pair
    in ~1.2us), then reciprocal + gpsimd partition_broadcast + DVE mul into
    A^T. The last head uses a shorter chain (reciprocal straight from PSUM,
    broadcast via a PE matmul with a ones stationary) since the PE idles
    there and the proj tail gates on it.
  - output projection: kt=5 (the last-normalized pair) is deferred behind
    other tiles' kt0-4 accumulation; proj tile 0's mains pre-run as pair-5
    filler. With the common all-zero bias the PSUM->SBUF y copy runs on
    the act engine (idle during proj; keeps the DVE off the psum-reuse
    WAR chain); a nonzero bias falls back to a DVE add against a
    pre-broadcast [128,C] bias tile.

The m (key/value position) axis is ordered [tokens(1024) | prefix(64)] —
softmax is permutation invariant, and this keeps every tile 128-aligned.
"""

import os
import sys

if os.environ.get("PRET_NOCACHE"):
    try:
        import jax
        jax.config.update("jax_enable_compilation_cache", False)
    except Exception:
        pass

for _p in ("/opt/trn_rl_repo", "/root/.axon_site/_ro/trn_rl_repo"):
    if os.path.isdir(_p) and _p not in sys.path:
        sys.path.insert(0, _p)

import numpy as np

import contextlib
import itertools

import concourse.bass as bass
import concourse.mybir as mybir
import concourse.tile as tile
from concourse import bacc
from concourse import bass_utils
from concourse import library_config
from concourse.bass_utils import run_bass_kernel_spmd

F32 = mybir.dt.float32
R32 = mybir.dt.float32r
BF16 = mybir.dt.bfloat16
EXP = mybir.ActivationFunctionType.Exp

B, N, C, H, D, P = 8, 1024, 768, 12, 64, 64
M = N + P            # 1088 key/value positions, tokens first then prefix
KT = C // 128        # 6 contraction k-tiles
NT = N // 128        # 8 token tiles
MT = N // 128        # 8 full (token) m-tiles; prefix handled separately
NP = H // 2          # 6 head pairs
WR = 16 * 128        # wq_rest columns per kt: q1-5 | k1-5 | v(6)
SCALE = D ** -0.5

REPEAT = int(os.environ.get("PRET_REPEAT", "1"))
DT = {"fp32r": R32, "bf16": BF16}[os.environ.get("PRET_DT", "bf16")]
POLICY = os.environ.get("PRET_POLICY", "3")
GDO = os.environ.get("PRET_GDO", "0")
# filler chunks pulled per pipeline slot in the attention steady loop
PULL = int(os.environ.get("PRET_PULL", "1"))
LEAD_N = int(os.environ.get("PRET_LEAD", "2"))
EB = int(os.environ.get("PRET_EB", "3"))


@contextlib.contextmanager
def _ldw_opt():
    # Patches walrus flags: --policy (post-scheduler) and --enable-ldw-opt
    # (fp32r only; bf16 matmuls emit explicit InstLdweights that walrus
    # rejects under ldw-opt).
    if DT != R32 and POLICY == "0" and GDO == "0":
        yield
        return
    orig = bass_utils.run_command

    def patched(argv, **kw):
        out = []
        for a in argv:
            if a == "--enable-ldw-opt=false" and DT == R32:
                a = "--enable-ldw-opt=true"
            elif a == "--policy=0":
                a = f"--policy={POLICY}"
            out.append(a)
        if GDO != "0" and out and out[0].endswith("walrus_driver"):
            out.append(f"--global-dma-ordering-optimization={GDO}")
        if os.environ.get("PRET_SDMA_SP") and out and out[0].endswith("walrus_driver"):
            out = [a.replace("--assign-static-dmas-to-sp=false",
                             "--assign-static-dmas-to-sp=true") for a in out]
        return orig(out, **kw)

    bass_utils.run_command = patched
    try:
        yield
    finally:
        bass_utils.run_command = orig


def build_nc(repeat=REPEAT, zero_bias=bool(int(os.environ.get("PRET_ZB", "1")))):
    nc = bacc.Bacc("TRN2", target_bir_lowering=False, debug=False,
                   dynamic_dma_scratch_size=int(os.environ.get("PRET_DDS", "16384")),
                   use_seq_codegen=bool(int(os.environ.get("PRET_SEQCG", "0"))),
                   num_swdge_queues=int(os.environ.get("PRET_SWQ", "1")))

    # all inputs partition-major so each is a single DMA
    xT0 = nc.dram_tensor("xT0", (128, N), DT, kind="ExternalInput")
    xTr = nc.dram_tensor("xTr", (128, (KT - 1) * N), DT, kind="ExternalInput")
    # bootstrap W columns: per kt [q-pair0 (128) | k-pair0 (128)]
    wqb = nc.dram_tensor("wqb", (128, KT * 256), DT, kind="ExternalInput")
    # the rest: per kt [q pairs 1-5 (640) | k pairs 1-5 (640) | v (768)]
    wqr = nc.dram_tensor("wqr", (128, KT * WR), DT, kind="ExternalInput")
    wp = nc.dram_tensor("wp", (128, KT * C), DT, kind="ExternalInput")
    # block-diag packed prefix k per head pair: [[k_even^T, 0], [0, k_odd^T]]
    kpk = nc.dram_tensor("kpk", (128, NP * 128), DT, kind="ExternalInput")
    # packed prefix v (+ ones col) per pair/head, other head's rows zeroed
    vpk = nc.dram_tensor("vpk", (128, NP * 2 * (D + 1)), DT, kind="ExternalInput")
    bb = nc.dram_tensor("bb", (128, C), DT, kind="ExternalInput")  # bias bcast
    y = nc.dram_tensor("y", (NT, 128, C), DT, kind="ExternalOutput")
    DEBUG = bool(os.environ.get("PRET_DEBUG"))
    if DEBUG:
        qdbg = nc.dram_tensor("qdbg", (128, KT * N), DT, kind="ExternalOutput")
        kdbg = nc.dram_tensor("kdbg", (128, KT * N), DT, kind="ExternalOutput")
        adbg = nc.dram_tensor("adbg", (128, KT * N), DT, kind="ExternalOutput")
        vdbg = nc.dram_tensor("vdbg", (128, MT * H * 65), DT, kind="ExternalOutput")

    with tile.TileContext(nc) as tc:
        with (
            nc.allow_low_precision(reason="bf16/fp32r matmul operands, f32 accum"),
            tc.tile_pool(name="const", bufs=1) as const_pool,
            tc.tile_pool(name="data", bufs=1) as data_pool,
            tc.tile_pool(name="work", bufs=2) as work_pool,
            tc.tile_pool(name="psum", bufs=2, space="PSUM") as pp,
        ):
            # ---- persistent SBUF tensors ----
            q_sb = data_pool.tile([128, KT, N], DT)          # q^T, pair rows
            ktok_sb = data_pool.tile([128, KT, N], DT)       # k^T tokens, pair rows
            v_sb = data_pool.tile([128, MT, H * 65], DT)     # v + ones cols
            a_sb = data_pool.tile([128, KT, N], DT)          # A^T attn out
            # input tiles double-buffered across repeat bodies: body i+1's
            # DMAs land in the other parity set, so they never WAR-wait on
            # body i's late filler reads (which run ~95% into the body)
            kpk_sbs = [data_pool.tile([128, NP, 128], DT, name=f"kpk{p}") for p in range(2)]
            vpk_sbs = [data_pool.tile([128, NP, 2, D + 1], DT, name=f"vpk{p}") for p in range(2)]
            wp_sbs = [data_pool.tile([128, KT, C], DT, name=f"wp{p}") for p in range(2)]
            bb_sbs = [data_pool.tile([128, C], DT, name=f"bb{p}") for p in range(2)]
            xT0_sbs = [data_pool.tile([128, N], DT, name=f"xT0{p}") for p in range(2)]
            xTr_sbs = [data_pool.tile([128, KT - 1, N], DT, name=f"xTr{p}") for p in range(2)]
            wqb_sbs = [data_pool.tile([128, KT, 2, 128], DT, name=f"wqb{p}") for p in range(2)]
            wqr_sbs = [data_pool.tile([128, KT, WR], DT, name=f"wqr{p}") for p in range(2)]

            nc.gpsimd.load_library(library_config.attn)
            # memset can't write fp32r/bf16; stage ones in f32 and copy.
            # col 64 of each head block of v must be 1.0 (softmax denoms)
            ones_f32 = const_pool.tile([128, 128], F32)
            nc.vector.memset(ones_f32[:], 1.0)
            ones_dt = const_pool.tile([1, 128], DT)
            nc.vector.tensor_copy(ones_dt[:], ones_f32[0:1, :])
            v_ones = v_sb.rearrange("p m (h e) -> p m h e", e=65)[:, :, :, 64]
            nc.vector.tensor_copy(
                v_ones, ones_f32[:, 0 : MT * H].rearrange("p (m h) -> p m h", m=MT)
            )

            def emit_body(par=0):
              kpk_sb, vpk_sb, wp_sb, bb_sb = (
                  kpk_sbs[par], vpk_sbs[par], wp_sbs[par], bb_sbs[par])
              xT0_sb, xTr_sb, wqb_sb, wqr_sb = (
                  xT0_sbs[par], xTr_sbs[par], wqb_sbs[par], wqr_sbs[par])

              def xt(kt):
                  return xT0_sb[:] if kt == 0 else xTr_sb[:, kt - 1, :]

              def wq_col(kt, mt):
                  # stationary W^T column block for qk out-tile mt (0..11)
                  if mt == 0:
                      return wqb_sb[:, kt, 0, :]
                  if mt == KT:
                      return wqb_sb[:, kt, 1, :]
                  if mt < KT:   # q pairs 1..5
                      c0 = (mt - 1) * 128
                  else:         # k pairs 1..5
                      c0 = 640 + (mt - KT - 1) * 128
                  return wqr_sb[:, kt, c0 : c0 + 128]

              def wv_col(kt, j0, j1):
                  return wqr_sb[:, kt, 1280 + j0 : 1280 + j1]

              # ---- DMA: one transfer per tensor; nothing on the act queue ----
              nc.sync.dma_start(xT0_sb[:], xT0[:])
              nc.sync.dma_start(wqb_sb.rearrange("p t h c -> p (t h c)"), wqb[:])
              nc.sync.dma_start(
                  xTr_sb[:, 0:2, :].rearrange("p t n -> p (t n)"),
                  xTr[:, 0 : 2 * N],
              )
              nc.sync.dma_start(
                  xTr_sb[:, 2:, :].rearrange("p t n -> p (t n)"),
                  xTr[:, 2 * N :],
              )
              nc.gpsimd.dma_start(kpk_sb.rearrange("p t m -> p (t m)"), kpk[:])
              nc.gpsimd.dma_start(vpk_sb.rearrange("p t h e -> p (t h e)"), vpk[:])
              nc.gpsimd.dma_start(wqr_sb.rearrange("p t c -> p (t c)"), wqr[:])
              nc.gpsimd.dma_start(wp_sb.rearrange("p t c -> p (t c)"), wp[:])
              if not zero_bias:
                  nc.gpsimd.dma_start(bb_sb[:], bb[:])

              tile_seq = itertools.count()

              def qk_emitter(mt, tag="f", bufs=1):
                  # generator: one 512-col matmul chunk per next(); the
                  # PSUM->SBUF copy runs when the generator is exhausted
                  ps = pp.tile([128, N], F32, tag=tag, bufs=bufs, name=f"ps_qk{mt}")
                  k0 = next(tile_seq)
                  for i in range(KT):
                      kt = (k0 + i) % KT
                      for nb in range(2):
                          nc.tensor.matmul(
                              ps[:, nb * 512 : (nb + 1) * 512],
                              wq_col(kt, mt),
                              xt(kt)[:, nb * 512 : (nb + 1) * 512],
                              start=(i == 0),
                              stop=(i == KT - 1),
                          )
                          yield
                  if mt < KT:
                      nc.vector.tensor_copy(q_sb[:, mt, :], ps[:])
                  else:
                      nc.vector.tensor_copy(ktok_sb[:, mt - KT, :], ps[:])

              def drain(gen):
                  for _ in gen:
                      pass

              def emit_v_tile(nt, tag="f", bufs=1):
                  # n-tile nt of natural v: stationary x^T, moving W_v^T
                  ps = pp.tile([128, 1024], F32, tag=tag, bufs=bufs, name=f"ps_v{nt}")
                  k0 = next(tile_seq)
                  for i in range(KT):
                      kt = (k0 + i) % KT
                      for j0, j1 in ((0, 512), (512, C)):
                          nc.tensor.matmul(
                              ps[:, j0:j1],
                              xt(kt)[:, nt * 128 : (nt + 1) * 128],
                              wv_col(kt, j0, j1),
                              start=(i == 0),
                              stop=(i == KT - 1),
                          )
                  dst = v_sb.rearrange("p m (h e) -> p m h e", e=65)[:, nt, :, 0:D]
                  nc.vector.tensor_copy(dst, ps[:, 0:C].rearrange("p (h d) -> p h d", h=H))

              # ---- attention helpers ----
              def s_pref(t):
                  # packed prefix S for both heads of pair t: one [128,1024]
                  # tile (rows 0:64 = even head's 64 prefix keys, 64:128 odd)
                  ps = pp.tile([128, N], F32, tag="s", name=f"ps_sp{t}")
                  for nb in range(2):
                      nc.tensor.matmul(
                          ps[:, nb * 512 : (nb + 1) * 512],
                          kpk_sb[:, t, :],
                          q_sb[:, t, nb * 512 : (nb + 1) * 512],
                          start=True,
                          stop=True,
                      )
                  ep = work_pool.tile([128, N], DT, tag="ep", bufs=2, name=f"ep{t}")
                  nc.scalar.activation(ep[:], ps[:], EXP, scale=SCALE)
                  return ep

              def emit_s(h, mt):
                  t, r = h // 2, (h % 2) * 64
                  ps = pp.tile([128, N], F32, tag="s", name=f"ps_s{h}_{mt}")
                  for nb in range(2):
                      nc.tensor.matmul(
                          ps[:, nb * 512 : (nb + 1) * 512],
                          ktok_sb[r : r + D, t, mt * 128 : (mt + 1) * 128],
                          q_sb[r : r + D, t, nb * 512 : (nb + 1) * 512],
                          start=True,
                          stop=True,
                      )
                  e_sb = work_pool.tile([128, N], DT, tag="e", bufs=EB, name=f"e{h}_{mt}")
                  nc.scalar.activation(e_sb[:], ps[:], EXP, scale=SCALE)
                  return e_sb

              def emit_pv_pref(h, po, ep):
                  # prefix PV: stationary has the other head's rows zeroed
                  t = h // 2
                  for nb in range(2):
                      nc.tensor.matmul(
                          po[:, nb * 512 : (nb + 1) * 512],
                          vpk_sb[:, t, h % 2, :],
                          ep[:, nb * 512 : (nb + 1) * 512],
                          start=True,
                          stop=False,
                      )

              def emit_pv(h, mt, po, e_sb):
                  for nb in range(2):
                      nc.tensor.matmul(
                          po[:, nb * 512 : (nb + 1) * 512],
                          v_sb[:, mt, h * 65 : (h + 1) * 65],
                          e_sb[:, nb * 512 : (nb + 1) * 512],
                          start=False,
                          stop=(mt == MT - 1),
                      )

              def normalize(h, po):
                  # Copy po out first ([65,1024] DVE copy) so the single 'o'
                  # PSUM buffer frees in ~1.2us instead of holding through the
                  # full recip->broadcast->mul chain (~3.7us).
                  t, r = h // 2, (h % 2) * 64
                  o_sb = work_pool.tile([65, N], F32, tag="oc", bufs=2, name=f"oc{h}")
                  nc.vector.tensor_copy(o_sb[:], po[:])
                  r_sb = work_pool.tile([1, N], F32, tag="r", name=f"r{h}")
                  nc.vector.reciprocal(r_sb[:], o_sb[64:65, :])
                  rb_sb = work_pool.tile([64, N], F32, tag="rb", name=f"rb{h}")
                  nc.gpsimd.partition_broadcast(rb_sb[:], r_sb[:])
                  nc.vector.tensor_mul(a_sb[r : r + 64, t, :], o_sb[0:64, :], rb_sb[:])

              def normalize_last(h, po):
                  # short-latency tail for the final head: reciprocal straight
                  # from PSUM (bf16 out), broadcast via a PE ones-matmul (the
                  # PE idles here; saves the ~1.5us gpsimd hop), multiply
                  # straight from PSUM.
                  t, r = h // 2, (h % 2) * 64
                  r_bf = work_pool.tile([1, N], DT, tag="r2", bufs=1, name=f"r2{h}")
                  nc.vector.reciprocal(r_bf[:], po[64:65, :])
                  o_sb = work_pool.tile([65, N], F32, tag="oc", bufs=2, name=f"oc{h}")
                  nc.vector.tensor_copy(o_sb[0:64, :], po[0:64, :])
                  rb_ps = pp.tile([128, N], F32, tag="s", name=f"rb_ps{h}")
                  for nb in range(2):
                      nc.tensor.matmul(
                          rb_ps[0:64, nb * 512 : (nb + 1) * 512],
                          ones_dt[0:1, 0:64],
                          r_bf[0:1, nb * 512 : (nb + 1) * 512],
                          start=True,
                          stop=True,
                      )
                  nc.vector.tensor_mul(
                      a_sb[r : r + 64, t, :], o_sb[0:64, :], rb_ps[0:64, :]
                  )

              def pull(stream, n=PULL):
                  for _ in range(n):
                      if next(stream, StopIteration) is StopIteration:
                          return

              def vtag(nt):
                  # alternate v tiles between the 'f' and 's' psum tags so the
                  # copy-drain of tile nt-1 never gates tile nt's first matmul
                  return ("f", 1) if nt % 2 == 0 else ("s", 2)

              streams = {0: itertools.chain(qk_emitter(1), qk_emitter(KT + 1))}
              for t in range(1, NP - 1):
                  streams[t] = itertools.chain(
                      qk_emitter(t + 1), qk_emitter(KT + t + 1)
                  )

              # ---- pair 0 bootstrap: k0/q0 up front, v tiles inside head 0 ----
              drain(qk_emitter(KT + 0, tag="s", bufs=2))   # k pair 0
              drain(qk_emitter(0, tag="s", bufs=2))        # q pair 0
              eps = {0: s_pref(0)}
              h = 0
              po = pp.tile([65, N], F32, tag="o", bufs=1, name="ps_o0")
              emit_v_tile(0, *vtag(0))
              es0 = [emit_s(h, 0), emit_s(h, 1)]
              emit_pv_pref(h, po, eps[0])
              emit_pv(h, 0, po, es0[0])
              for mt in range(2, MT):
                  emit_v_tile(mt - 1, *vtag(mt - 1))
                  es0.append(emit_s(h, mt))
                  emit_pv(h, mt - 1, po, es0[mt - 1])
              emit_v_tile(MT - 1, *vtag(MT - 1))
              pull(streams[0], 2)
              emit_pv(h, MT - 1, po, es0[MT - 1])
              pull(streams[0], 2)
              normalize(h, po)
              pull(streams[0], 2)

              # proj psum tiles: allocated by chunk generators, finished later
              pys = {}

              def proj_main_chunks(nt, tag, bufs):
                  # kt = 0..4 accumulation of proj tile nt (kt=5 needs the
                  # last pair's A and is deferred to proj_finish)
                  py = pp.tile([128, 1024], F32, tag=tag, bufs=bufs,
                               name=f"ps_y{nt}")
                  pys[nt] = py
                  for kt in range(KT - 1):
                      for j0, j1 in ((0, 512), (512, C)):
                          nc.tensor.matmul(
                              py[:, j0:j1],
                              a_sb[:, kt, nt * 128 : (nt + 1) * 128],
                              wp_sb[:, kt, j0:j1],
                              start=(kt == 0),
                              stop=False,
                          )
                          yield

              # ---- heads 1..11: flat cross-head software pipeline ----
              LEAD = LEAD_N
              flat = [(hh, mm) for hh in range(1, H) for mm in range(MT)]
              es = {}
              pos = {}
              # pair 5 has no next pair to produce; pre-run proj tile 0's
              # kt0-4 chunks there instead (they only need pairs 0..4)
              streams[NP - 1] = proj_main_chunks(0, "f", 1)

              def s_step(i):
                  if i >= len(flat):
                      return
                  hs, ms = flat[i]
                  if ms == 0 and hs % 2 == 0:
                      # the S stream is about to read pair hs//2's q/k tiles:
                      # force any unfinished producer chunks (and their
                      # PSUM->SBUF copies) to emit BEFORE the first read
                      if hs // 2 - 1 in streams:
                          drain(streams[hs // 2 - 1])
                      eps[hs // 2] = s_pref(hs // 2)
                  es[(hs, ms)] = emit_s(hs, ms)

              # prime the S stream
              for i in range(LEAD):
                  s_step(i)
              for i, (hh, mm) in enumerate(flat):
                  stream = streams[hh // 2]
                  s_step(i + LEAD)
                  pull(stream)
                  if mm == 0:
                      po = pp.tile([65, N], F32, tag="o", bufs=1, name=f"ps_o{hh}")
                      pos[hh] = po
                      emit_pv_pref(hh, po, eps[hh // 2])
                  emit_pv(hh, mm, po, es.pop((hh, mm)))
                  pull(stream)
                  if mm == MT - 1:
                      if hh == H - 1:
                          normalize_last(hh, pos.pop(hh))
                      else:
                          normalize(hh, pos.pop(hh))
                      if hh % 2 == 1:
                          drain(streams[hh // 2])

              # ---- output projection + bias (bias via DVE add) ----
              # kt=5 (head pair 5, the last to normalize) is deferred ~2
              # tiles behind the kt=0..4 accumulation so the last normalize
              # hides behind other proj work. Tile 0's mains already ran as
              # pair-5 filler. Tag sequence keeps every psum-buf reuse behind
              # the finish that drains it.
              def proj_finish(nt):
                  py = pys.pop(nt)
                  kt = KT - 1
                  for j0, j1 in ((0, 512), (512, C)):
                      nc.tensor.matmul(
                          py[:, j0:j1],
                          a_sb[:, kt, nt * 128 : (nt + 1) * 128],
                          wp_sb[:, kt, j0:j1],
                          start=False,
                          stop=True,
                      )
                  y_sb = work_pool.tile([128, C], DT, tag="y", name=f"y{nt}")
                  if zero_bias:
                      nc.scalar.activation(
                          y_sb[:], py[:, 0:C], mybir.ActivationFunctionType.Copy
                      )
                  else:
                      nc.vector.tensor_add(y_sb[:], py[:, 0:C], bb_sb[:])
                  nc.sync.dma_start(y[nt], y_sb[:])

              for step in (
                  ("m", 1, "s", 2), ("m", 2, "s", 2), ("f", 0),
                  ("m", 3, "f", 1), ("f", 1), ("m", 4, "s", 2), ("f", 2),
                  ("m", 5, "s", 2), ("f", 3), ("m", 6, "f", 1), ("f", 4),
                  ("m", 7, "s", 2), ("f", 5), ("f", 6), ("f", 7),
              ):
                  if step[0] == "m":
                      drain(proj_main_chunks(step[1], step[2], step[3]))
                  else:
                      proj_finish(step[1])

              if DEBUG:
                  nc.sync.dma_start(qdbg[:], q_sb.rearrange("p t n -> p (t n)"))
                  nc.sync.dma_start(kdbg[:], ktok_sb.rearrange("p t n -> p (t n)"))
                  nc.sync.dma_start(adbg[:], a_sb.rearrange("p t n -> p (t n)"))
                  nc.sync.dma_start(vdbg[:], v_sb.rearrange("p m c -> p (m c)"))

            for rep in range(repeat):
                emit_body(rep % 2)

    nc.compile()
    return nc


def make_in_maps(x, prompt, W_qkv, W_proj, b_proj):
    if DT == BF16:
        import ml_dtypes
        cast = lambda a: np.asarray(a, dtype=ml_dtypes.bfloat16)
    else:
        cast = lambda a: np.asarray(a, dtype=np.float32)
    x, prompt, W_qkv, W_proj, b_proj = map(cast, (x, prompt, W_qkv, W_proj, b_proj))
    npdt = x.dtype
    # W^T with contraction on partitions: [KT, 128, 3C] -> partition-major
    wq_t = W_qkv.T.reshape(KT, 128, 3 * C).transpose(1, 0, 2)  # [128, KT, 3C]
    # bootstrap: q-pair0 cols (0:128) and k-pair0 cols (768:896)
    wqb_h = np.ascontiguousarray(
        np.stack([wq_t[:, :, 0:128], wq_t[:, :, 768:896]], axis=2).reshape(
            128, KT * 256
        )
    )
    # rest: q pairs 1-5 | k pairs 1-5 | v
    wqr_h = np.ascontiguousarray(
        np.concatenate(
            [wq_t[:, :, 128:768], wq_t[:, :, 896:1536], wq_t[:, :, 1536:2304]],
            axis=2,
        ).reshape(128, KT * WR)
    )
    wp_h = np.ascontiguousarray(
        W_proj.T.reshape(KT, 128, C).transpose(1, 0, 2).reshape(128, KT * C)
    )
    bb_h = np.ascontiguousarray(np.broadcast_to(b_proj.reshape(1, C), (128, C)))
    maps = []
    for b in range(B):
        kp = prompt[b, 0]  # (P, H, D)
        vp = prompt[b, 1]  # (P, H, D)
        kpk_h = np.zeros((128, NP, 128), dtype=npdt)
        vpk_h = np.zeros((128, NP, 2, D + 1), dtype=npdt)
        for t in range(NP):
            kpk_h[0:D, t, 0:P] = kp[:, 2 * t, :].T
            kpk_h[D:128, t, P:128] = kp[:, 2 * t + 1, :].T
            vpk_h[0:P, t, 0, 0:D] = vp[:, 2 * t, :]
            vpk_h[0:P, t, 0, D] = 1.0
            vpk_h[P:128, t, 1, 0:D] = vp[:, 2 * t + 1, :]
            vpk_h[P:128, t, 1, D] = 1.0
        xt_t = x[b].T.reshape(KT, 128, N)  # [KT, 128, N]
        maps.append(
            {
                "xT0": np.ascontiguousarray(xt_t[0]),
                "xTr": np.ascontiguousarray(
                    xt_t[1:].transpose(1, 0, 2).reshape(128, (KT - 1) * N)
                ),
                "wqb": wqb_h,
                "wqr": wqr_h,
                "wp": wp_h,
                "kpk": np.ascontiguousarray(kpk_h.reshape(128, NP * 128)),
                "vpk": np.ascontiguousarray(vpk_h.reshape(128, NP * 2 * (D + 1))),
                "bb": bb_h,
            }
        )
    return maps


def kernel(x, prompt, W_qkv, W_proj, b_proj, **run_kwargs):
    x = np.asarray(x, dtype=np.float32)
    prompt = np.asarray(prompt, dtype=np.float32)
    W_qkv = np.asarray(W_qkv, dtype=np.float32)
    W_proj = np.asarray(W_proj, dtype=np.float32)
    b_proj = np.asarray(b_proj, dtype=np.float32)

    nc = build_nc(repeat=1, zero_bias=bool(np.all(b_proj == 0)))
    in_maps = make_in_maps(x, prompt, W_qkv, W_proj, b_proj)
    with _ldw_opt():
        res = run_bass_kernel_spmd(nc, in_maps, core_ids=list(range(B)), **run_kwargs)
    out = np.stack(
        [np.asarray(res.results[b]["y"], dtype=np.float32).reshape(N, C) for b in range(B)]
    )
    if run_kwargs:
        kernel.last_results = res
    return out


# revision 38
# speedup vs baseline: 1.0252x; 1.0088x over previous
"""PreT_Attention (prefix-KV multi-head attention) on 8 Trainium2 NeuronCores.

Strategy: pure data parallelism — batch B=8 is split 1 element per core; the
qkv/proj weights are replicated. No collectives. Host-side numpy does layout
marshalling only (transposes / reshapes), all FLOPs run on device.

Device kernel (per core), bf16 operands with f32 PSUM accumulation. The
schedule is a single software-pipelined stream built around the PE engine
(the roofline engine at ~150us of matmul column-cycles):

  - DMA: few large partition-major transfers (issue cost on a sequencer is
    ~1us each, so tensors are packed host-side for one DMA apiece). The
    bootstrap weights (k0/q0 column blocks of W_qkv) are a separate small
    tensor so the first matmuls start ~3us in. Nothing is issued on the
    scalar queue — DMA issues there would delay the exp stream.
  - qkv projection tiles (q^T / k^T pair tiles, natural-v tiles) from
    pre-transposed x^T and W^T with the contraction dim on partitions. Only
    pair 0's q/k tiles are produced up front; v tiles interleave with head
    0's S/PV loop; later q/k pair tiles are emitted as chunk-granular FILLER
    inside the attention pipeline so the PE stays busy during act-bound
    stretches.
  - attention: flat cross-head software pipeline — the S/exp stream runs 2
    steps ahead of the PV stream (across head boundaries), so the act engine
    never faces a cold e0 at head entry. The prefix-KV part of S is computed
    once per head pair from a block-diagonal packed k-prefix (one exp for
    both heads); softmax denominators ride as a 65th ones-column per head
    block in v (zeroed on the other head's rows in the packed v-prefix).
  - normalization: DVE copies PSUM out fast (frees the single 'o' bank pair
    in ~1.2us), then reciprocal + gpsimd partition_broadcast + DVE mul into
    A^T. The last head uses a shorter chain (reciprocal straight from PSUM,
    broadcast via a PE matmul with a ones stationary) since the PE idles
    there and the proj tail gates on it.
  - output projection: kt=5 (the last-normalized pair) is deferred behind
    other tiles' kt0-4 accumulation; proj tile 0's mains pre-run as pair-5
    filler. With the common all-zero bias the PSUM->SBUF y copy runs on
    the act engine (idle during proj; keeps the DVE off the psum-reuse
    WAR chain); a nonzero bias falls back to a DVE add against a
    pre-broadcast [128,C] bias tile.

The m (key/value position) axis is ordered [tokens(1024) | prefix(64)] —
softmax is permutation invariant, and this keeps every tile 128-aligned.
"""

import os
import sys

if os.environ.get("PRET_NOCACHE"):
    try:
        import jax
        jax.config.update("jax_enable_compilation_cache", False)
    except Exception:
        pass

for _p in ("/opt/trn_rl_repo", "/root/.axon_site/_ro/trn_rl_repo"):
    if os.path.isdir(_p) and _p not in sys.path:
        sys.path.insert(0, _p)

import numpy as np

import contextlib
import itertools

import concourse.bass as bass
import concourse.mybir as mybir
import concourse.tile as tile
from concourse import bacc
from concourse import bass_utils
from concourse import library_config
from concourse.bass_utils import run_bass_kernel_spmd

F32 = mybir.dt.float32
R32 = mybir.dt.float32r
BF16 = mybir.dt.bfloat16
EXP = mybir.ActivationFunctionType.Exp

B, N, C, H, D, P = 8, 1024, 768, 12, 64, 64
M = N + P            # 1088 key/value positions, tokens first then prefix
KT = C // 128        # 6 contraction k-tiles
NT = N // 128        # 8 token tiles
MT = N // 128        # 8 full (token) m-tiles; prefix handled separately
NP = H // 2          # 6 head pairs
WR = 16 * 128        # wq_rest columns per kt: q1-5 | k1-5 | v(6)
SCALE = D ** -0.5

REPEAT = int(os.environ.get("PRET_REPEAT", "1"))
DT = {"fp32r": R32, "bf16": BF16}[os.environ.get("PRET_DT", "bf16")]
POLICY = os.environ.get("PRET_POLICY", "3")
GDO = os.environ.get("PRET_GDO", "0")
# filler chunks pulled per pipeline slot in the attention steady loop
PULL = int(os.environ.get("PRET_PULL", "1"))
LEAD_N = int(os.environ.get("PRET_LEAD", "2"))
EB = int(os.environ.get("PRET_EB", "3"))


@contextlib.contextmanager
def _ldw_opt():
    # Patches walrus flags: --policy (post-scheduler) and --enable-ldw-opt
    # (fp32r only; bf16 matmuls emit explicit InstLdweights that walrus
    # rejects under ldw-opt).
    if DT != R32 and POLICY == "0" and GDO == "0":
        yield
        return
    orig = bass_utils.run_command

    def patched(argv, **kw):
        out = []
        for a in argv:
            if a == "--enable-ldw-opt=false" and DT == R32:
                a = "--enable-ldw-opt=true"
            elif a == "--policy=0":
                a = f"--policy={POLICY}"
            out.append(a)
        if GDO != "0" and out and out[0].endswith("walrus_driver"):
            out.append(f"--global-dma-ordering-optimization={GDO}")
        if os.environ.get("PRET_SDMA_SP") and out and out[0].endswith("walrus_driver"):
            out = [a.replace("--assign-static-dmas-to-sp=false",
                             "--assign-static-dmas-to-sp=true") for a in out]
        return orig(out, **kw)

    bass_utils.run_command = patched
    try:
        yield
    finally:
        bass_utils.run_command = orig


def build_nc(repeat=REPEAT, zero_bias=bool(int(os.environ.get("PRET_ZB", "1")))):
    nc = bacc.Bacc("TRN2", target_bir_lowering=False, debug=False,
                   dynamic_dma_scratch_size=int(os.environ.get("PRET_DDS", "16384")),
                   use_seq_codegen=bool(int(os.environ.get("PRET_SEQCG", "0"))),
                   num_swdge_queues=int(os.environ.get("PRET_SWQ", "1")),
                   ultra=bool(int(os.environ.get("PRET_ULTRA", "0"))),
                   monotonic_sem_count=int(os.environ.get("PRET_MSC", "1")))

    # all inputs partition-major so each is a single DMA
    xT0 = nc.dram_tensor("xT0", (128, N), DT, kind="ExternalInput")
    xTr = nc.dram_tensor("xTr", (128, (KT - 1) * N), DT, kind="ExternalInput")
    # bootstrap W columns: per kt [q-pair0 (128) | k-pair0 (128)]
    wqb = nc.dram_tensor("wqb", (128, KT * 256), DT, kind="ExternalInput")
    # the rest: per kt [q pairs 1-5 (640) | k pairs 1-5 (640) | v (768)]
    wqr = nc.dram_tensor("wqr", (128, KT * WR), DT, kind="ExternalInput")
    wp = nc.dram_tensor("wp", (128, KT * C), DT, kind="ExternalInput")
    # block-diag packed prefix k per head pair: [[k_even^T, 0], [0, k_odd^T]]
    kpk = nc.dram_tensor("kpk", (128, NP * 128), DT, kind="ExternalInput")
    # packed prefix v (+ ones col) per pair/head, other head's rows zeroed
    vpk = nc.dram_tensor("vpk", (128, NP * 2 * (D + 1)), DT, kind="ExternalInput")
    bb = nc.dram_tensor("bb", (128, C), DT, kind="ExternalInput")  # bias bcast
    y = nc.dram_tensor("y", (NT, 128, C), DT, kind="ExternalOutput")
    DEBUG = bool(os.environ.get("PRET_DEBUG"))
    if DEBUG:
        qdbg = nc.dram_tensor("qdbg", (128, KT * N), DT, kind="ExternalOutput")
        kdbg = nc.dram_tensor("kdbg", (128, KT * N), DT, kind="ExternalOutput")
        adbg = nc.dram_tensor("adbg", (128, KT * N), DT, kind="ExternalOutput")
        vdbg = nc.dram_tensor("vdbg", (128, MT * H * 65), DT, kind="ExternalOutput")

    with tile.TileContext(nc) as tc:
        with (
            nc.allow_low_precision(reason="bf16/fp32r matmul operands, f32 accum"),
            tc.tile_pool(name="const", bufs=1) as const_pool,
            tc.tile_pool(name="data", bufs=1) as data_pool,
            tc.tile_pool(name="work", bufs=2) as work_pool,
            tc.tile_pool(name="psum", bufs=2, space="PSUM") as pp,
        ):
            # ---- persistent SBUF tensors ----
            q_sb = data_pool.tile([128, KT, N], DT)          # q^T, pair rows
            ktok_sb = data_pool.tile([128, KT, N], DT)       # k^T tokens, pair rows
            v_sb = data_pool.tile([128, MT, H * 65], DT)     # v + ones cols
            a_sb = data_pool.tile([128, KT, N], DT)          # A^T attn out
            # input tiles double-buffered across repeat bodies: body i+1's
            # DMAs land in the other parity set, so they never WAR-wait on
            # body i's late filler reads (which run ~95% into the body)
            kpk_sbs = [data_pool.tile([128, NP, 128], DT, name=f"kpk{p}") for p in range(2)]
            vpk_sbs = [data_pool.tile([128, NP, 2, D + 1], DT, name=f"vpk{p}") for p in range(2)]
            wp_sbs = [data_pool.tile([128, KT, C], DT, name=f"wp{p}") for p in range(2)]
            bb_sbs = [data_pool.tile([128, C], DT, name=f"bb{p}") for p in range(2)]
            xT0_sbs = [data_pool.tile([128, N], DT, name=f"xT0{p}") for p in range(2)]
            xTr_sbs = [data_pool.tile([128, KT - 1, N], DT, name=f"xTr{p}") for p in range(2)]
            wqb_sbs = [data_pool.tile([128, KT, 2, 128], DT, name=f"wqb{p}") for p in range(2)]
            wqr_sbs = [data_pool.tile([128, KT, WR], DT, name=f"wqr{p}") for p in range(2)]

            nc.gpsimd.load_library(library_config.attn)
            # memset can't write fp32r/bf16; stage ones in f32 and copy.
            # col 64 of each head block of v must be 1.0 (softmax denoms)
            ones_f32 = const_pool.tile([128, 128], F32)
            nc.vector.memset(ones_f32[:], 1.0)
            ones_dt = const_pool.tile([1, 128], DT)
            nc.vector.tensor_copy(ones_dt[:], ones_f32[0:1, :])
            v_ones = v_sb.rearrange("p m (h e) -> p m h e", e=65)[:, :, :, 64]
            nc.vector.tensor_copy(
                v_ones, ones_f32[:, 0 : MT * H].rearrange("p (m h) -> p m h", m=MT)
            )

            def emit_body(par=0):
              kpk_sb, vpk_sb, wp_sb, bb_sb = (
                  kpk_sbs[par], vpk_sbs[par], wp_sbs[par], bb_sbs[par])
              xT0_sb, xTr_sb, wqb_sb, wqr_sb = (
                  xT0_sbs[par], xTr_sbs[par], wqb_sbs[par], wqr_sbs[par])

              def xt(kt):
                  return xT0_sb[:] if kt == 0 else xTr_sb[:, kt - 1, :]

              def wq_col(kt, mt):
                  # stationary W^T column block for qk out-tile mt (0..11)
                  if mt == 0:
                      return wqb_sb[:, kt, 0, :]
                  if mt == KT:
                      return wqb_sb[:, kt, 1, :]
                  if mt < KT:   # q pairs 1..5
                      c0 = (mt - 1) * 128
                  else:         # k pairs 1..5
                      c0 = 640 + (mt - KT - 1) * 128
                  return wqr_sb[:, kt, c0 : c0 + 128]

              def wv_col(kt, j0, j1):
                  return wqr_sb[:, kt, 1280 + j0 : 1280 + j1]

              # ---- DMA: one transfer per tensor; nothing on the act queue ----
              nc.sync.dma_start(xT0_sb[:], xT0[:])
              nc.sync.dma_start(wqb_sb.rearrange("p t h c -> p (t h c)"), wqb[:])
              nc.sync.dma_start(
                  xTr_sb[:, 0:2, :].rearrange("p t n -> p (t n)"),
                  xTr[:, 0 : 2 * N],
              )
              nc.sync.dma_start(
                  xTr_sb[:, 2:, :].rearrange("p t n -> p (t n)"),
                  xTr[:, 2 * N :],
              )
              nc.gpsimd.dma_start(kpk_sb.rearrange("p t m -> p (t m)"), kpk[:])
              nc.gpsimd.dma_start(vpk_sb.rearrange("p t h e -> p (t h e)"), vpk[:])
              nc.gpsimd.dma_start(wqr_sb.rearrange("p t c -> p (t c)"), wqr[:])
              nc.gpsimd.dma_start(wp_sb.rearrange("p t c -> p (t c)"), wp[:])
              if not zero_bias:
                  nc.gpsimd.dma_start(bb_sb[:], bb[:])

              tile_seq = itertools.count()

              def qk_emitter(mt, tag="f", bufs=1):
                  # generator: one 512-col matmul chunk per next(); the
                  # PSUM->SBUF copy runs when the generator is exhausted
                  ps = pp.tile([128, N], F32, tag=tag, bufs=bufs, name=f"ps_qk{mt}")
                  k0 = next(tile_seq)
                  for i in range(KT):
                      kt = (k0 + i) % KT
                      for nb in range(2):
                          nc.tensor.matmul(
                              ps[:, nb * 512 : (nb + 1) * 512],
                              wq_col(kt, mt),
                              xt(kt)[:, nb * 512 : (nb + 1) * 512],
                              start=(i == 0),
                              stop=(i == KT - 1),
                          )
                          yield
                  if mt < KT:
                      nc.vector.tensor_copy(q_sb[:, mt, :], ps[:])
                  else:
                      nc.vector.tensor_copy(ktok_sb[:, mt - KT, :], ps[:])

              def drain(gen):
                  for _ in gen:
                      pass

              def emit_v_tile(nt, tag="f", bufs=1):
                  # n-tile nt of natural v: stationary x^T, moving W_v^T
                  ps = pp.tile([128, 1024], F32, tag=tag, bufs=bufs, name=f"ps_v{nt}")
                  k0 = next(tile_seq)
                  for i in range(KT):
                      kt = (k0 + i) % KT
                      for j0, j1 in ((0, 512), (512, C)):
                          nc.tensor.matmul(
                              ps[:, j0:j1],
                              xt(kt)[:, nt * 128 : (nt + 1) * 128],
                              wv_col(kt, j0, j1),
                              start=(i == 0),
                              stop=(i == KT - 1),
                          )
                  dst = v_sb.rearrange("p m (h e) -> p m h e", e=65)[:, nt, :, 0:D]
                  nc.vector.tensor_copy(dst, ps[:, 0:C].rearrange("p (h d) -> p h d", h=H))

              # ---- attention helpers ----
              def s_pref(t):
                  # packed prefix S for both heads of pair t: one [128,1024]
                  # tile (rows 0:64 = even head's 64 prefix keys, 64:128 odd)
                  ps = pp.tile([128, N], F32, tag="s", name=f"ps_sp{t}")
                  for nb in range(2):
                      nc.tensor.matmul(
                          ps[:, nb * 512 : (nb + 1) * 512],
                          kpk_sb[:, t, :],
                          q_sb[:, t, nb * 512 : (nb + 1) * 512],
                          start=True,
                          stop=True,
                      )
                  ep = work_pool.tile([128, N], DT, tag="ep", bufs=2, name=f"ep{t}")
                  nc.scalar.activation(ep[:], ps[:], EXP, scale=SCALE)
                  return ep

              def emit_s(h, mt):
                  t, r = h // 2, (h % 2) * 64
                  ps = pp.tile([128, N], F32, tag="s", name=f"ps_s{h}_{mt}")
                  for nb in range(2):
                      nc.tensor.matmul(
                          ps[:, nb * 512 : (nb + 1) * 512],
                          ktok_sb[r : r + D, t, mt * 128 : (mt + 1) * 128],
                          q_sb[r : r + D, t, nb * 512 : (nb + 1) * 512],
                          start=True,
                          stop=True,
                      )
                  e_sb = work_pool.tile([128, N], DT, tag="e", bufs=EB, name=f"e{h}_{mt}")
                  nc.scalar.activation(e_sb[:], ps[:], EXP, scale=SCALE)
                  return e_sb

              def emit_pv_pref(h, po, ep):
                  # prefix PV: stationary has the other head's rows zeroed
                  t = h // 2
                  for nb in range(2):
                      nc.tensor.matmul(
                          po[:, nb * 512 : (nb + 1) * 512],
                          vpk_sb[:, t, h % 2, :],
                          ep[:, nb * 512 : (nb + 1) * 512],
                          start=True,
                          stop=False,
                      )

              def emit_pv(h, mt, po, e_sb):
                  for nb in range(2):
                      nc.tensor.matmul(
                          po[:, nb * 512 : (nb + 1) * 512],
                          v_sb[:, mt, h * 65 : (h + 1) * 65],
                          e_sb[:, nb * 512 : (nb + 1) * 512],
                          start=False,
                          stop=(mt == MT - 1),
                      )

              def normalize(h, po):
                  # Copy po out first ([65,1024] DVE copy) so the single 'o'
                  # PSUM buffer frees in ~1.2us instead of holding through the
                  # full recip->broadcast->mul chain (~3.7us).
                  t, r = h // 2, (h % 2) * 64
                  o_sb = work_pool.tile([65, N], F32, tag="oc", bufs=2, name=f"oc{h}")
                  nc.vector.tensor_copy(o_sb[:], po[:])
                  r_sb = work_pool.tile([1, N], F32, tag="r", name=f"r{h}")
                  nc.vector.reciprocal(r_sb[:], o_sb[64:65, :])
                  rb_sb = work_pool.tile([64, N], F32, tag="rb", name=f"rb{h}")
                  nc.gpsimd.partition_broadcast(rb_sb[:], r_sb[:])
                  nc.vector.tensor_mul(a_sb[r : r + 64, t, :], o_sb[0:64, :], rb_sb[:])

              def normalize_last(h, po):
                  # short-latency tail for the final head: reciprocal straight
                  # from PSUM (bf16 out), broadcast via a PE ones-matmul (the
                  # PE idles here; saves the ~1.5us gpsimd hop), multiply
                  # straight from PSUM.
                  t, r = h // 2, (h % 2) * 64
                  r_bf = work_pool.tile([1, N], DT, tag="r2", bufs=1, name=f"r2{h}")
                  nc.vector.reciprocal(r_bf[:], po[64:65, :])
                  o_sb = work_pool.tile([65, N], F32, tag="oc", bufs=2, name=f"oc{h}")
                  nc.vector.tensor_copy(o_sb[0:64, :], po[0:64, :])
                  rb_ps = pp.tile([128, N], F32, tag="s", name=f"rb_ps{h}")
                  for nb in range(2):
                      nc.tensor.matmul(
                          rb_ps[0:64, nb * 512 : (nb + 1) * 512],
                          ones_dt[0:1, 0:64],
                          r_bf[0:1, nb * 512 : (nb + 1) * 512],
                          start=True,
                          stop=True,
                      )
                  nc.vector.tensor_mul(
                      a_sb[r : r + 64, t, :], o_sb[0:64, :], rb_ps[0:64, :]
                  )

              def pull(stream, n=PULL):
                  for _ in range(n):
                      if next(stream, StopIteration) is StopIteration:
                          return

              def vtag(nt):
                  # alternate v tiles between the 'f' and 's' psum tags so the
                  # copy-drain of tile nt-1 never gates tile nt's first matmul
                  return ("f", 1) if nt % 2 == 0 else ("s", 2)

              streams = {0: itertools.chain(qk_emitter(1), qk_emitter(KT + 1))}
              for t in range(1, NP - 1):
                  streams[t] = itertools.chain(
                      qk_emitter(t + 1), qk_emitter(KT + t + 1)
                  )

              # ---- pair 0 bootstrap: k0/q0 up front, v tiles inside head 0 ----
              drain(qk_emitter(KT + 0, tag="s", bufs=2))   # k pair 0
              drain(qk_emitter(0, tag="s", bufs=2))        # q pair 0
              eps = {0: s_pref(0)}
              h = 0
              po = pp.tile([65, N], F32, tag="o", bufs=1, name="ps_o0")
              emit_v_tile(0, *vtag(0))
              es0 = [emit_s(h, 0), emit_s(h, 1)]
              emit_pv_pref(h, po, eps[0])
              emit_pv(h, 0, po, es0[0])
              for mt in range(2, MT):
                  emit_v_tile(mt - 1, *vtag(mt - 1))
                  es0.append(emit_s(h, mt))
                  emit_pv(h, mt - 1, po, es0[mt - 1])
              emit_v_tile(MT - 1, *vtag(MT - 1))
              pull(streams[0], 2)
              emit_pv(h, MT - 1, po, es0[MT - 1])
              pull(streams[0], 2)
              normalize(h, po)
              pull(streams[0], 2)

              # proj psum tiles: allocated by chunk generators, finished later
              pys = {}

              def proj_main_chunks(nt, tag, bufs):
                  # kt = 0..4 accumulation of proj tile nt (kt=5 needs the
                  # last pair's A and is deferred to proj_finish)
                  py = pp.tile([128, 1024], F32, tag=tag, bufs=bufs,
                               name=f"ps_y{nt}")
                  pys[nt] = py
                  for kt in range(KT - 1):
                      for j0, j1 in ((0, 512), (512, C)):
                          nc.tensor.matmul(
                              py[:, j0:j1],
                              a_sb[:, kt, nt * 128 : (nt + 1) * 128],
                              wp_sb[:, kt, j0:j1],
                              start=(kt == 0),
                              stop=False,
                          )
                          yield

              # ---- heads 1..11: flat cross-head software pipeline ----
              LEAD = LEAD_N
              flat = [(hh, mm) for hh in range(1, H) for mm in range(MT)]
              es = {}
              pos = {}
              # pair 5 has no next pair to produce; pre-run proj tile 0's
              # kt0-4 chunks there instead (they only need pairs 0..4)
              streams[NP - 1] = proj_main_chunks(0, "f", 1)

              def s_step(i):
                  if i >= len(flat):
                      return
                  hs, ms = flat[i]
                  if ms == 0 and hs % 2 == 0:
                      # the S stream is about to read pair hs//2's q/k tiles:
                      # force any unfinished producer chunks (and their
                      # PSUM->SBUF copies) to emit BEFORE the first read
                      if hs // 2 - 1 in streams:
                          drain(streams[hs // 2 - 1])
                      eps[hs // 2] = s_pref(hs // 2)
                  es[(hs, ms)] = emit_s(hs, ms)

              # prime the S stream
              for i in range(LEAD):
                  s_step(i)
              for i, (hh, mm) in enumerate(flat):
                  stream = streams[hh // 2]
                  s_step(i + LEAD)
                  pull(stream)
                  if mm == 0:
                      po = pp.tile([65, N], F32, tag="o", bufs=1, name=f"ps_o{hh}")
                      pos[hh] = po
                      emit_pv_pref(hh, po, eps[hh // 2])
                  emit_pv(hh, mm, po, es.pop((hh, mm)))
                  pull(stream)
                  if mm == MT - 1:
                      if hh == H - 1:
                          normalize_last(hh, pos.pop(hh))
                      else:
                          normalize(hh, pos.pop(hh))
                      if hh % 2 == 1:
                          drain(streams[hh // 2])

              # ---- output projection + bias (bias via DVE add) ----
              # kt=5 (head pair 5, the last to normalize) is deferred ~2
              # tiles behind the kt=0..4 accumulation so the last normalize
              # hides behind other proj work. Tile 0's mains already ran as
              # pair-5 filler. Tag sequence keeps every psum-buf reuse behind
              # the finish that drains it.
              def proj_finish(nt):
                  py = pys.pop(nt)
                  kt = KT - 1
                  for j0, j1 in ((0, 512), (512, C)):
                      nc.tensor.matmul(
                          py[:, j0:j1],
                          a_sb[:, kt, nt * 128 : (nt + 1) * 128],
                          wp_sb[:, kt, j0:j1],
                          start=False,
                          stop=True,
                      )
                  y_sb = work_pool.tile([128, C], DT, tag="y", name=f"y{nt}")
                  if zero_bias:
                      nc.scalar.activation(
                          y_sb[:], py[:, 0:C], mybir.ActivationFunctionType.Copy
                      )
                  else:
                      nc.vector.tensor_add(y_sb[:], py[:, 0:C], bb_sb[:])
                  nc.sync.dma_start(y[nt], y_sb[:])

              for step in (
                  ("m", 1, "s", 2), ("m", 2, "s", 2), ("f", 0),
                  ("m", 3, "f", 1), ("f", 1), ("m", 4, "s", 2), ("f", 2),
                  ("m", 5, "s", 2), ("f", 3), ("m", 6, "f", 1), ("f", 4),
                  ("m", 7, "s", 2), ("f", 5), ("f", 6), ("f", 7),
              ):
                  if step[0] == "m":
                      drain(proj_main_chunks(step[1], step[2], step[3]))
                  else:
                      proj_finish(step[1])

              if DEBUG:
                  nc.sync.dma_start(qdbg[:], q_sb.rearrange("p t n -> p (t n)"))
                  nc.sync.dma_start(kdbg[:], ktok_sb.rearrange("p t n -> p (t n)"))
                  nc.sync.dma_start(adbg[:], a_sb.rearrange("p t n -> p (t n)"))
                  nc.sync.dma_start(vdbg[:], v_sb.rearrange("p m c -> p (m c)"))

            for rep in range(repeat):
                emit_body(rep % 2)

    nc.compile()
    return nc


def make_in_maps(x, prompt, W_qkv, W_proj, b_proj):
    if DT == BF16:
        import ml_dtypes
        cast = lambda a: np.asarray(a, dtype=ml_dtypes.bfloat16)
    else:
        cast = lambda a: np.asarray(a, dtype=np.float32)
    x, prompt, W_qkv, W_proj, b_proj = map(cast, (x, prompt, W_qkv, W_proj, b_proj))
    npdt = x.dtype
    # W^T with contraction on partitions: [KT, 128, 3C] -> partition-major
    wq_t = W_qkv.T.reshape(KT, 128, 3 * C).transpose(1, 0, 2)  # [128, KT, 3C]
    # bootstrap: q-pair0 cols (0:128) and k-pair0 cols (768:896)
    wqb_h = np.ascontiguousarray(
        np.stack([wq_t[:, :, 0:128], wq_t[:, :, 768:896]], axis=2).reshape(
            128, KT * 256
        )
    )
    # rest: q pairs 1-5 | k pairs 1-5 | v
    wqr_h = np.ascontiguousarray(
        np.concatenate(
            [wq_t[:, :, 128:768], wq_t[:, :, 896:1536], wq_t[:, :, 1536:2304]],
            axis=2,
        ).reshape(128, KT * WR)
    )
    wp_h = np.ascontiguousarray(
        W_proj.T.reshape(KT, 128, C).transpose(1, 0, 2).reshape(128, KT * C)
    )
    bb_h = np.ascontiguousarray(np.broadcast_to(b_proj.reshape(1, C), (128, C)))
    maps = []
    for b in range(B):
        kp = prompt[b, 0]  # (P, H, D)
        vp = prompt[b, 1]  # (P, H, D)
        kpk_h = np.zeros((128, NP, 128), dtype=npdt)
        vpk_h = np.zeros((128, NP, 2, D + 1), dtype=npdt)
        for t in range(NP):
            kpk_h[0:D, t, 0:P] = kp[:, 2 * t, :].T
            kpk_h[D:128, t, P:128] = kp[:, 2 * t + 1, :].T
            vpk_h[0:P, t, 0, 0:D] = vp[:, 2 * t, :]
            vpk_h[0:P, t, 0, D] = 1.0
            vpk_h[P:128, t, 1, 0:D] = vp[:, 2 * t + 1, :]
            vpk_h[P:128, t, 1, D] = 1.0
        xt_t = x[b].T.reshape(KT, 128, N)  # [KT, 128, N]
        maps.append(
            {
                "xT0": np.ascontiguousarray(xt_t[0]),
                "xTr": np.ascontiguousarray(
                    xt_t[1:].transpose(1, 0, 2).reshape(128, (KT - 1) * N)
                ),
                "wqb": wqb_h,
                "wqr": wqr_h,
                "wp": wp_h,
                "kpk": np.ascontiguousarray(kpk_h.reshape(128, NP * 128)),
                "vpk": np.ascontiguousarray(vpk_h.reshape(128, NP * 2 * (D + 1))),
                "bb": bb_h,
            }
        )
    return maps


def kernel(x, prompt, W_qkv, W_proj, b_proj, **run_kwargs):
    x = np.asarray(x, dtype=np.float32)
    prompt = np.asarray(prompt, dtype=np.float32)
    W_qkv = np.asarray(W_qkv, dtype=np.float32)
    W_proj = np.asarray(W_proj, dtype=np.float32)
    b_proj = np.asarray(b_proj, dtype=np.float32)

    nc = build_nc(repeat=1, zero_bias=bool(np.all(b_proj == 0)))
    in_maps = make_in_maps(x, prompt, W_qkv, W_proj, b_proj)
    with _ldw_opt():
        res = run_bass_kernel_spmd(nc, in_maps, core_ids=list(range(B)), **run_kwargs)
    out = np.stack(
        [np.asarray(res.results[b]["y"], dtype=np.float32).reshape(N, C) for b in range(B)]
    )
    if run_kwargs:
        kernel.last_results = res
    return out


# revision 39
# speedup vs baseline: 1.0384x; 1.0128x over previous
"""PreT_Attention (prefix-KV multi-head attention) on 8 Trainium2 NeuronCores.

Strategy: pure data parallelism — batch B=8 is split 1 element per core; the
qkv/proj weights are replicated. No collectives. Host-side numpy does layout
marshalling only (transposes / reshapes), all FLOPs run on device.

Device kernel (per core), bf16 operands with f32 PSUM accumulation. The
schedule is a single software-pipelined stream built around the PE engine
(the roofline engine at ~150us of matmul column-cycles):

  - DMA: few large partition-major transfers (issue cost on a sequencer is
    ~1us each, so tensors are packed host-side for one DMA apiece). The
    bootstrap weights (k0/q0 column blocks of W_qkv) are a separate small
    tensor so the first matmuls start ~3us in. Nothing is issued on the
    scalar queue — DMA issues there would delay the exp stream.
  - qkv projection tiles (q^T / k^T pair tiles, natural-v tiles) from
    pre-transposed x^T and W^T with the contraction dim on partitions. Only
    pair 0's q/k tiles are produced up front; v tiles interleave with head
    0's S/PV loop; later q/k pair tiles are emitted as chunk-granular FILLER
    inside the attention pipeline so the PE stays busy during act-bound
    stretches.
  - attention: flat cross-head software pipeline — the S/exp stream runs 2
    steps ahead of the PV stream (across head boundaries), so the act engine
    never faces a cold e0 at head entry. The prefix-KV part of S is computed
    once per head pair from a block-diagonal packed k-prefix (one exp for
    both heads); softmax denominators ride as a 65th ones-column per head
    block in v (zeroed on the other head's rows in the packed v-prefix).
  - normalization: DVE copies PSUM out fast (frees the single 'o' bank pair
    in ~1.2us), then reciprocal + gpsimd partition_broadcast + DVE mul into
    A^T. The last head uses a shorter chain (reciprocal straight from PSUM,
    broadcast via a PE matmul with a ones stationary) since the PE idles
    there and the proj tail gates on it.
  - output projection: kt=5 (the last-normalized pair) is deferred behind
    other tiles' kt0-4 accumulation; proj tile 0's mains pre-run as pair-5
    filler. With the common all-zero bias the PSUM->SBUF y copy runs on
    the act engine (idle during proj; keeps the DVE off the psum-reuse
    WAR chain); a nonzero bias falls back to a DVE add against a
    pre-broadcast [128,C] bias tile.

The m (key/value position) axis is ordered [tokens(1024) | prefix(64)] —
softmax is permutation invariant, and this keeps every tile 128-aligned.
"""

import os
import sys

if os.environ.get("PRET_NOCACHE"):
    try:
        import jax
        jax.config.update("jax_enable_compilation_cache", False)
    except Exception:
        pass

for _p in ("/opt/trn_rl_repo", "/root/.axon_site/_ro/trn_rl_repo"):
    if os.path.isdir(_p) and _p not in sys.path:
        sys.path.insert(0, _p)

import numpy as np

import contextlib
import itertools

import concourse.bass as bass
import concourse.mybir as mybir
import concourse.tile as tile
from concourse import bacc
from concourse import bass_utils
from concourse import library_config
from concourse.bass_utils import run_bass_kernel_spmd

F32 = mybir.dt.float32
R32 = mybir.dt.float32r
BF16 = mybir.dt.bfloat16
EXP = mybir.ActivationFunctionType.Exp

B, N, C, H, D, P = 8, 1024, 768, 12, 64, 64
M = N + P            # 1088 key/value positions, tokens first then prefix
KT = C // 128        # 6 contraction k-tiles
NT = N // 128        # 8 token tiles
MT = N // 128        # 8 full (token) m-tiles; prefix handled separately
NP = H // 2          # 6 head pairs
WR = 16 * 128        # wq_rest columns per kt: q1-5 | k1-5 | v(6)
SCALE = D ** -0.5

REPEAT = int(os.environ.get("PRET_REPEAT", "1"))
DT = {"fp32r": R32, "bf16": BF16}[os.environ.get("PRET_DT", "bf16")]
POLICY = os.environ.get("PRET_POLICY", "3")
GDO = os.environ.get("PRET_GDO", "0")
# filler chunks pulled per pipeline slot in the attention steady loop
PULL = int(os.environ.get("PRET_PULL", "1"))
LEAD_N = int(os.environ.get("PRET_LEAD", "2"))
EB = int(os.environ.get("PRET_EB", "3"))


@contextlib.contextmanager
def _ldw_opt():
    # Patches walrus flags: --policy (post-scheduler) and --enable-ldw-opt
    # (fp32r only; bf16 matmuls emit explicit InstLdweights that walrus
    # rejects under ldw-opt).
    if DT != R32 and POLICY == "0" and GDO == "0":
        yield
        return
    orig = bass_utils.run_command

    def patched(argv, **kw):
        out = []
        for a in argv:
            if a == "--enable-ldw-opt=false" and DT == R32:
                a = "--enable-ldw-opt=true"
            elif a == "--policy=0":
                a = f"--policy={POLICY}"
            out.append(a)
        if GDO != "0" and out and out[0].endswith("walrus_driver"):
            out.append(f"--global-dma-ordering-optimization={GDO}")
        if os.environ.get("PRET_SDMA_SP") and out and out[0].endswith("walrus_driver"):
            out = [a.replace("--assign-static-dmas-to-sp=false",
                             "--assign-static-dmas-to-sp=true") for a in out]
        return orig(out, **kw)

    bass_utils.run_command = patched
    try:
        yield
    finally:
        bass_utils.run_command = orig


def build_nc(repeat=REPEAT, zero_bias=bool(int(os.environ.get("PRET_ZB", "1")))):
    nc = bacc.Bacc("TRN2", target_bir_lowering=False, debug=False,
                   dynamic_dma_scratch_size=int(os.environ.get("PRET_DDS", "16384")),
                   use_seq_codegen=bool(int(os.environ.get("PRET_SEQCG", "0"))),
                   num_swdge_queues=int(os.environ.get("PRET_SWQ", "1")),
                   ultra=bool(int(os.environ.get("PRET_ULTRA", "0"))),
                   monotonic_sem_count=int(os.environ.get("PRET_MSC", "1")))

    # all inputs partition-major so each is a single DMA
    xT0 = nc.dram_tensor("xT0", (128, N), DT, kind="ExternalInput")
    xTr = nc.dram_tensor("xTr", (128, (KT - 1) * N), DT, kind="ExternalInput")
    # bootstrap W columns: per kt [q-pair0 (128) | k-pair0 (128)]
    wqb = nc.dram_tensor("wqb", (128, KT * 256), DT, kind="ExternalInput")
    # the rest: per kt [q pairs 1-5 (640) | k pairs 1-5 (640) | v (768)]
    wqr = nc.dram_tensor("wqr", (128, KT * WR), DT, kind="ExternalInput")
    wp = nc.dram_tensor("wp", (128, KT * C), DT, kind="ExternalInput")
    # block-diag packed prefix k per head pair: [[k_even^T, 0], [0, k_odd^T]]
    kpk = nc.dram_tensor("kpk", (128, NP * 128), DT, kind="ExternalInput")
    # packed prefix v (+ ones col) per pair/head, other head's rows zeroed
    vpk = nc.dram_tensor("vpk", (128, NP * 2 * (D + 1)), DT, kind="ExternalInput")
    bb = nc.dram_tensor("bb", (128, C), DT, kind="ExternalInput")  # bias bcast
    y = nc.dram_tensor("y", (NT, 128, C), DT, kind="ExternalOutput")
    DEBUG = bool(os.environ.get("PRET_DEBUG"))
    if DEBUG:
        qdbg = nc.dram_tensor("qdbg", (128, KT * N), DT, kind="ExternalOutput")
        kdbg = nc.dram_tensor("kdbg", (128, KT * N), DT, kind="ExternalOutput")
        adbg = nc.dram_tensor("adbg", (128, KT * N), DT, kind="ExternalOutput")
        vdbg = nc.dram_tensor("vdbg", (128, MT * H * 65), DT, kind="ExternalOutput")

    with tile.TileContext(nc) as tc:
        with (
            nc.allow_low_precision(reason="bf16/fp32r matmul operands, f32 accum"),
            tc.tile_pool(name="const", bufs=1) as const_pool,
            tc.tile_pool(name="data", bufs=1) as data_pool,
            tc.tile_pool(name="work", bufs=2) as work_pool,
            tc.tile_pool(name="psum", bufs=2, space="PSUM") as pp,
        ):
            # ---- persistent SBUF tensors ----
            q_sb = data_pool.tile([128, KT, N], DT)          # q^T, pair rows
            ktok_sb = data_pool.tile([128, KT, N], DT)       # k^T tokens, pair rows
            v_sb = data_pool.tile([128, MT, H * 65], DT)     # v + ones cols
            a_sb = data_pool.tile([128, KT, N], DT)          # A^T attn out
            # input tiles double-buffered across repeat bodies: body i+1's
            # DMAs land in the other parity set, so they never WAR-wait on
            # body i's late filler reads (which run ~95% into the body)
            kpk_sbs = [data_pool.tile([128, NP, 128], DT, name=f"kpk{p}") for p in range(2)]
            vpk_sbs = [data_pool.tile([128, NP, 2, D + 1], DT, name=f"vpk{p}") for p in range(2)]
            wp_sbs = [data_pool.tile([128, KT, C], DT, name=f"wp{p}") for p in range(2)]
            bb_sbs = [data_pool.tile([128, C], DT, name=f"bb{p}") for p in range(2)]
            xT0_sbs = [data_pool.tile([128, N], DT, name=f"xT0{p}") for p in range(2)]
            xTr_sbs = [data_pool.tile([128, KT - 1, N], DT, name=f"xTr{p}") for p in range(2)]
            wqb_sbs = [data_pool.tile([128, KT, 2, 128], DT, name=f"wqb{p}") for p in range(2)]
            wqr_sbs = [data_pool.tile([128, KT, WR], DT, name=f"wqr{p}") for p in range(2)]

            nc.gpsimd.load_library(library_config.attn)
            # memset can't write fp32r/bf16; stage ones in f32 and copy.
            # col 64 of each head block of v must be 1.0 (softmax denoms)
            ones_f32 = const_pool.tile([128, 128], F32)
            nc.vector.memset(ones_f32[:], 1.0)
            ones_dt = const_pool.tile([1, 128], DT)
            nc.vector.tensor_copy(ones_dt[:], ones_f32[0:1, :])
            v_ones = v_sb.rearrange("p m (h e) -> p m h e", e=65)[:, :, :, 64]
            nc.vector.tensor_copy(
                v_ones, ones_f32[:, 0 : MT * H].rearrange("p (m h) -> p m h", m=MT)
            )

            def emit_body(par=0):
              kpk_sb, vpk_sb, wp_sb, bb_sb = (
                  kpk_sbs[par], vpk_sbs[par], wp_sbs[par], bb_sbs[par])
              xT0_sb, xTr_sb, wqb_sb, wqr_sb = (
                  xT0_sbs[par], xTr_sbs[par], wqb_sbs[par], wqr_sbs[par])

              def xt(kt):
                  return xT0_sb[:] if kt == 0 else xTr_sb[:, kt - 1, :]

              def wq_col(kt, mt):
                  # stationary W^T column block for qk out-tile mt (0..11)
                  if mt == 0:
                      return wqb_sb[:, kt, 0, :]
                  if mt == KT:
                      return wqb_sb[:, kt, 1, :]
                  if mt < KT:   # q pairs 1..5
                      c0 = (mt - 1) * 128
                  else:         # k pairs 1..5
                      c0 = 640 + (mt - KT - 1) * 128
                  return wqr_sb[:, kt, c0 : c0 + 128]

              def wv_col(kt, j0, j1):
                  return wqr_sb[:, kt, 1280 + j0 : 1280 + j1]

              # ---- DMA: one transfer per tensor; nothing on the act queue ----
              nc.sync.dma_start(xT0_sb[:], xT0[:])
              nc.sync.dma_start(wqb_sb.rearrange("p t h c -> p (t h c)"), wqb[:])
              nc.sync.dma_start(
                  xTr_sb[:, 0:2, :].rearrange("p t n -> p (t n)"),
                  xTr[:, 0 : 2 * N],
              )
              nc.sync.dma_start(
                  xTr_sb[:, 2:, :].rearrange("p t n -> p (t n)"),
                  xTr[:, 2 * N :],
              )
              nc.gpsimd.dma_start(kpk_sb.rearrange("p t m -> p (t m)"), kpk[:])
              nc.gpsimd.dma_start(vpk_sb.rearrange("p t h e -> p (t h e)"), vpk[:])
              nc.gpsimd.dma_start(wqr_sb.rearrange("p t c -> p (t c)"), wqr[:])
              nc.gpsimd.dma_start(wp_sb.rearrange("p t c -> p (t c)"), wp[:])
              if not zero_bias:
                  nc.gpsimd.dma_start(bb_sb[:], bb[:])

              tile_seq = itertools.count()

              def qk_emitter(mt, tag="f", bufs=1):
                  # generator: one 512-col matmul chunk per next(); the
                  # PSUM->SBUF copy runs when the generator is exhausted
                  ps = pp.tile([128, N], F32, tag=tag, bufs=bufs, name=f"ps_qk{mt}")
                  k0 = next(tile_seq)
                  for i in range(KT):
                      kt = (k0 + i) % KT
                      for nb in range(2):
                          nc.tensor.matmul(
                              ps[:, nb * 512 : (nb + 1) * 512],
                              wq_col(kt, mt),
                              xt(kt)[:, nb * 512 : (nb + 1) * 512],
                              start=(i == 0),
                              stop=(i == KT - 1),
                          )
                          yield
                  if mt < KT:
                      nc.vector.tensor_copy(q_sb[:, mt, :], ps[:])
                  else:
                      nc.vector.tensor_copy(ktok_sb[:, mt - KT, :], ps[:])

              def drain(gen):
                  for _ in gen:
                      pass

              def emit_v_tile(nt, tag="f", bufs=1):
                  # n-tile nt of natural v: stationary x^T, moving W_v^T
                  ps = pp.tile([128, 1024], F32, tag=tag, bufs=bufs, name=f"ps_v{nt}")
                  k0 = next(tile_seq)
                  for i in range(KT):
                      kt = (k0 + i) % KT
                      for j0, j1 in ((0, 512), (512, C)):
                          nc.tensor.matmul(
                              ps[:, j0:j1],
                              xt(kt)[:, nt * 128 : (nt + 1) * 128],
                              wv_col(kt, j0, j1),
                              start=(i == 0),
                              stop=(i == KT - 1),
                          )
                  dst = v_sb.rearrange("p m (h e) -> p m h e", e=65)[:, nt, :, 0:D]
                  nc.vector.tensor_copy(dst, ps[:, 0:C].rearrange("p (h d) -> p h d", h=H))

              # ---- attention helpers ----
              def s_pref(t):
                  # packed prefix S for both heads of pair t: one [128,1024]
                  # tile (rows 0:64 = even head's 64 prefix keys, 64:128 odd)
                  ps = pp.tile([128, N], F32, tag="s", name=f"ps_sp{t}")
                  for nb in range(2):
                      nc.tensor.matmul(
                          ps[:, nb * 512 : (nb + 1) * 512],
                          kpk_sb[:, t, :],
                          q_sb[:, t, nb * 512 : (nb + 1) * 512],
                          start=True,
                          stop=True,
                      )
                  ep = work_pool.tile([128, N], DT, tag="ep", bufs=2, name=f"ep{t}")
                  nc.scalar.activation(ep[:], ps[:], EXP, scale=SCALE)
                  return ep

              def emit_s(h, mt):
                  t, r = h // 2, (h % 2) * 64
                  ps = pp.tile([128, N], F32, tag="s", name=f"ps_s{h}_{mt}")
                  for nb in range(2):
                      nc.tensor.matmul(
                          ps[:, nb * 512 : (nb + 1) * 512],
                          ktok_sb[r : r + D, t, mt * 128 : (mt + 1) * 128],
                          q_sb[r : r + D, t, nb * 512 : (nb + 1) * 512],
                          start=True,
                          stop=True,
                      )
                  e_sb = work_pool.tile([128, N], DT, tag="e", bufs=EB, name=f"e{h}_{mt}")
                  nc.scalar.activation(e_sb[:], ps[:], EXP, scale=SCALE)
                  return e_sb

              def emit_pv_pref(h, po, ep):
                  # prefix PV: stationary has the other head's rows zeroed
                  t = h // 2
                  for nb in range(2):
                      nc.tensor.matmul(
                          po[:, nb * 512 : (nb + 1) * 512],
                          vpk_sb[:, t, h % 2, :],
                          ep[:, nb * 512 : (nb + 1) * 512],
                          start=True,
                          stop=False,
                      )

              def emit_pv(h, mt, po, e_sb):
                  for nb in range(2):
                      nc.tensor.matmul(
                          po[:, nb * 512 : (nb + 1) * 512],
                          v_sb[:, mt, h * 65 : (h + 1) * 65],
                          e_sb[:, nb * 512 : (nb + 1) * 512],
                          start=False,
                          stop=(mt == MT - 1),
                      )

              def normalize(h, po):
                  # Copy po out first ([65,1024] DVE copy) so the single 'o'
                  # PSUM buffer frees in ~1.2us instead of holding through the
                  # full recip->broadcast->mul chain (~3.7us).
                  t, r = h // 2, (h % 2) * 64
                  o_sb = work_pool.tile([65, N], F32, tag="oc", bufs=2, name=f"oc{h}")
                  nc.vector.tensor_copy(o_sb[:], po[:])
                  r_sb = work_pool.tile([1, N], F32, tag="r", name=f"r{h}")
                  nc.vector.reciprocal(r_sb[:], o_sb[64:65, :])
                  rb_sb = work_pool.tile([64, N], F32, tag="rb", name=f"rb{h}")
                  nc.gpsimd.partition_broadcast(rb_sb[:], r_sb[:])
                  nc.vector.tensor_mul(a_sb[r : r + 64, t, :], o_sb[0:64, :], rb_sb[:])

              def normalize_last(h, po):
                  # short-latency tail for the final head: reciprocal straight
                  # from PSUM (bf16 out), broadcast via a PE ones-matmul (the
                  # PE idles here; saves the ~1.5us gpsimd hop), multiply
                  # straight from PSUM.
                  # progressive per-nb chain: the first proj finishes only
                  # read the first columns of a, so releasing them early
                  # (range-tracked) starts the tail ~1.7us sooner
                  t, r = h // 2, (h % 2) * 64
                  r_bf = work_pool.tile([1, N], DT, tag="r2", bufs=1, name=f"r2{h}")
                  o_sb = work_pool.tile([65, N], F32, tag="oc", bufs=2, name=f"oc{h}")
                  rb_ps = pp.tile([128, N], F32, tag="s", name=f"rb_ps{h}")
                  for nb in range(2):
                      j0, j1 = nb * 512, (nb + 1) * 512
                      nc.vector.reciprocal(r_bf[0:1, j0:j1], po[64:65, j0:j1])
                      nc.vector.tensor_copy(o_sb[0:64, j0:j1], po[0:64, j0:j1])
                      nc.tensor.matmul(
                          rb_ps[0:64, j0:j1],
                          ones_dt[0:1, 0:64],
                          r_bf[0:1, j0:j1],
                          start=True,
                          stop=True,
                      )
                      nc.vector.tensor_mul(
                          a_sb[r : r + 64, t, j0:j1],
                          o_sb[0:64, j0:j1],
                          rb_ps[0:64, j0:j1],
                      )

              def pull(stream, n=PULL):
                  for _ in range(n):
                      if next(stream, StopIteration) is StopIteration:
                          return

              def vtag(nt):
                  # alternate v tiles between the 'f' and 's' psum tags so the
                  # copy-drain of tile nt-1 never gates tile nt's first matmul
                  return ("f", 1) if nt % 2 == 0 else ("s", 2)

              streams = {0: itertools.chain(qk_emitter(1), qk_emitter(KT + 1))}
              for t in range(1, NP - 1):
                  streams[t] = itertools.chain(
                      qk_emitter(t + 1), qk_emitter(KT + t + 1)
                  )

              # ---- pair 0 bootstrap: k0/q0 up front, v tiles inside head 0 ----
              drain(qk_emitter(KT + 0, tag="s", bufs=2))   # k pair 0
              drain(qk_emitter(0, tag="s", bufs=2))        # q pair 0
              eps = {0: s_pref(0)}
              h = 0
              po = pp.tile([65, N], F32, tag="o", bufs=1, name="ps_o0")
              emit_v_tile(0, *vtag(0))
              es0 = [emit_s(h, 0), emit_s(h, 1)]
              emit_pv_pref(h, po, eps[0])
              emit_pv(h, 0, po, es0[0])
              for mt in range(2, MT):
                  emit_v_tile(mt - 1, *vtag(mt - 1))
                  es0.append(emit_s(h, mt))
                  emit_pv(h, mt - 1, po, es0[mt - 1])
              emit_v_tile(MT - 1, *vtag(MT - 1))
              pull(streams[0], 2)
              emit_pv(h, MT - 1, po, es0[MT - 1])
              pull(streams[0], 2)
              normalize(h, po)
              pull(streams[0], 2)

              # proj psum tiles: allocated by chunk generators, finished later
              pys = {}

              def proj_main_chunks(nt, tag, bufs):
                  # kt = 0..4 accumulation of proj tile nt (kt=5 needs the
                  # last pair's A and is deferred to proj_finish)
                  py = pp.tile([128, 1024], F32, tag=tag, bufs=bufs,
                               name=f"ps_y{nt}")
                  pys[nt] = py
                  for kt in range(KT - 1):
                      for j0, j1 in ((0, 512), (512, C)):
                          nc.tensor.matmul(
                              py[:, j0:j1],
                              a_sb[:, kt, nt * 128 : (nt + 1) * 128],
                              wp_sb[:, kt, j0:j1],
                              start=(kt == 0),
                              stop=False,
                          )
                          yield

              # ---- heads 1..11: flat cross-head software pipeline ----
              LEAD = LEAD_N
              flat = [(hh, mm) for hh in range(1, H) for mm in range(MT)]
              es = {}
              pos = {}
              # pair 5 has no next pair to produce; pre-run proj tile 0's
              # kt0-4 chunks there instead (they only need pairs 0..4)
              streams[NP - 1] = proj_main_chunks(0, "f", 1)

              def s_step(i):
                  if i >= len(flat):
                      return
                  hs, ms = flat[i]
                  if ms == 0 and hs % 2 == 0:
                      # the S stream is about to read pair hs//2's q/k tiles:
                      # force any unfinished producer chunks (and their
                      # PSUM->SBUF copies) to emit BEFORE the first read
                      if hs // 2 - 1 in streams:
                          drain(streams[hs // 2 - 1])
                      eps[hs // 2] = s_pref(hs // 2)
                  es[(hs, ms)] = emit_s(hs, ms)

              # prime the S stream
              for i in range(LEAD):
                  s_step(i)
              for i, (hh, mm) in enumerate(flat):
                  stream = streams[hh // 2]
                  s_step(i + LEAD)
                  pull(stream)
                  if mm == 0:
                      po = pp.tile([65, N], F32, tag="o", bufs=1, name=f"ps_o{hh}")
                      pos[hh] = po
                      emit_pv_pref(hh, po, eps[hh // 2])
                  emit_pv(hh, mm, po, es.pop((hh, mm)))
                  pull(stream)
                  if mm == MT - 1:
                      if hh == H - 1:
                          normalize_last(hh, pos.pop(hh))
                      else:
                          normalize(hh, pos.pop(hh))
                      if hh % 2 == 1:
                          drain(streams[hh // 2])

              # ---- output projection + bias (bias via DVE add) ----
              # kt=5 (head pair 5, the last to normalize) is deferred ~2
              # tiles behind the kt=0..4 accumulation so the last normalize
              # hides behind other proj work. Tile 0's mains already ran as
              # pair-5 filler. Tag sequence keeps every psum-buf reuse behind
              # the finish that drains it.
              def proj_finish(nt):
                  py = pys.pop(nt)
                  kt = KT - 1
                  for j0, j1 in ((0, 512), (512, C)):
                      nc.tensor.matmul(
                          py[:, j0:j1],
                          a_sb[:, kt, nt * 128 : (nt + 1) * 128],
                          wp_sb[:, kt, j0:j1],
                          start=False,
                          stop=True,
                      )
                  y_sb = work_pool.tile([128, C], DT, tag="y", name=f"y{nt}")
                  if zero_bias:
                      nc.scalar.activation(
                          y_sb[:], py[:, 0:C], mybir.ActivationFunctionType.Copy
                      )
                  else:
                      nc.vector.tensor_add(y_sb[:], py[:, 0:C], bb_sb[:])
                  nc.sync.dma_start(y[nt], y_sb[:])

              for step in (
                  ("m", 1, "s", 2), ("m", 2, "s", 2), ("f", 0),
                  ("m", 3, "f", 1), ("f", 1), ("m", 4, "s", 2), ("f", 2),
                  ("m", 5, "s", 2), ("f", 3), ("m", 6, "f", 1), ("f", 4),
                  ("m", 7, "s", 2), ("f", 5), ("f", 6), ("f", 7),
              ):
                  if step[0] == "m":
                      drain(proj_main_chunks(step[1], step[2], step[3]))
                  else:
                      proj_finish(step[1])

              if DEBUG:
                  nc.sync.dma_start(qdbg[:], q_sb.rearrange("p t n -> p (t n)"))
                  nc.sync.dma_start(kdbg[:], ktok_sb.rearrange("p t n -> p (t n)"))
                  nc.sync.dma_start(adbg[:], a_sb.rearrange("p t n -> p (t n)"))
                  nc.sync.dma_start(vdbg[:], v_sb.rearrange("p m c -> p (m c)"))

            for rep in range(repeat):
                emit_body(rep % 2)

    nc.compile()
    return nc


def make_in_maps(x, prompt, W_qkv, W_proj, b_proj):
    if DT == BF16:
        import ml_dtypes
        cast = lambda a: np.asarray(a, dtype=ml_dtypes.bfloat16)
    else:
        cast = lambda a: np.asarray(a, dtype=np.float32)
    x, prompt, W_qkv, W_proj, b_proj = map(cast, (x, prompt, W_qkv, W_proj, b_proj))
    npdt = x.dtype
    # W^T with contraction on partitions: [KT, 128, 3C] -> partition-major
    wq_t = W_qkv.T.reshape(KT, 128, 3 * C).transpose(1, 0, 2)  # [128, KT, 3C]
    # bootstrap: q-pair0 cols (0:128) and k-pair0 cols (768:896)
    wqb_h = np.ascontiguousarray(
        np.stack([wq_t[:, :, 0:128], wq_t[:, :, 768:896]], axis=2).reshape(
            128, KT * 256
        )
    )
    # rest: q pairs 1-5 | k pairs 1-5 | v
    wqr_h = np.ascontiguousarray(
        np.concatenate(
            [wq_t[:, :, 128:768], wq_t[:, :, 896:1536], wq_t[:, :, 1536:2304]],
            axis=2,
        ).reshape(128, KT * WR)
    )
    wp_h = np.ascontiguousarray(
        W_proj.T.reshape(KT, 128, C).transpose(1, 0, 2).reshape(128, KT * C)
    )
    bb_h = np.ascontiguousarray(np.broadcast_to(b_proj.reshape(1, C), (128, C)))
    maps = []
    for b in range(B):
        kp = prompt[b, 0]  # (P, H, D)
        vp = prompt[b, 1]  # (P, H, D)
        kpk_h = np.zeros((128, NP, 128), dtype=npdt)
        vpk_h = np.zeros((128, NP, 2, D + 1), dtype=npdt)
        for t in range(NP):
            kpk_h[0:D, t, 0:P] = kp[:, 2 * t, :].T
            kpk_h[D:128, t, P:128] = kp[:, 2 * t + 1, :].T
            vpk_h[0:P, t, 0, 0:D] = vp[:, 2 * t, :]
            vpk_h[0:P, t, 0, D] = 1.0
            vpk_h[P:128, t, 1, 0:D] = vp[:, 2 * t + 1, :]
            vpk_h[P:128, t, 1, D] = 1.0
        xt_t = x[b].T.reshape(KT, 128, N)  # [KT, 128, N]
        maps.append(
            {
                "xT0": np.ascontiguousarray(xt_t[0]),
                "xTr": np.ascontiguousarray(
                    xt_t[1:].transpose(1, 0, 2).reshape(128, (KT - 1) * N)
                ),
                "wqb": wqb_h,
                "wqr": wqr_h,
                "wp": wp_h,
                "kpk": np.ascontiguousarray(kpk_h.reshape(128, NP * 128)),
                "vpk": np.ascontiguousarray(vpk_h.reshape(128, NP * 2 * (D + 1))),
                "bb": bb_h,
            }
        )
    return maps


def kernel(x, prompt, W_qkv, W_proj, b_proj, **run_kwargs):
    x = np.asarray(x, dtype=np.float32)
    prompt = np.asarray(prompt, dtype=np.float32)
    W_qkv = np.asarray(W_qkv, dtype=np.float32)
    W_proj = np.asarray(W_proj, dtype=np.float32)
    b_proj = np.asarray(b_proj, dtype=np.float32)

    nc = build_nc(repeat=1, zero_bias=bool(np.all(b_proj == 0)))
    in_maps = make_in_maps(x, prompt, W_qkv, W_proj, b_proj)
    with _ldw_opt():
        res = run_bass_kernel_spmd(nc, in_maps, core_ids=list(range(B)), **run_kwargs)
    out = np.stack(
        [np.asarray(res.results[b]["y"], dtype=np.float32).reshape(N, C) for b in range(B)]
    )
    if run_kwargs:
        kernel.last_results = res
    return out
